# revision 42
# baseline (speedup 1.0000x reference)
"""NNConv (gnn_message_passing) SPMD kernel for 8 trn2 NeuronCores.

Strategy (dst-sharded, both layers):
  - Each core owns a contiguous range of NLOC nodes (dst sharding). Edges are
    assigned to the core owning their dst.
  - msg = kron([1, ea], h_src) @ Wstack  (the NNConv per-edge weight matmul
    factorizes into a plain matmul over a 128-wide feature built from
    c = [1, ea0, ea1, ea2] outer h_src).
  - Layer 1 source features: host pre-expands x[src] into a [128, EP1] bf16
    stream (pure indexing/layout), streamed sequentially from DRAM — no
    device-side gather. L1 edges are sorted by (dst window, dst), minimizing
    aggregation one-hot pairs.
  - Layer 2 source features: DMA transpose-gather (256B rows, bf16,
    replicated 4x) from an SBUF-resident table built from the allgathered
    compact h1. L2 edges sorted by (src-quarter, dst window, dst).
  - Aggregation (segment sum over dst): one-hot PE matmuls into PSUM-resident
    per-window accumulators (window = 128 dst nodes), fused with the root-term
    matmul and ReLU.
  - One AllGather (compact h1, bf16) between the layers.
  - Edge layout is made identical across cores via shared R-tables
    (cell counts padded to the max over cores), so a single SPMD program
    works for all 8 cores.
"""

import sys

if '/opt/trn_rl_repo' not in sys.path:
    sys.path.insert(0, '/opt/trn_rl_repo')

from contextlib import ExitStack

import ml_dtypes
import numpy as np

import concourse.bacc as bacc
import concourse.bass as bass
from concourse import mybir, tile
from concourse.bass_utils import run_bass_kernel_spmd
from concourse import library_config

BF16 = ml_dtypes.bfloat16
AF = mybir.ActivationFunctionType
ALU = mybir.AluOpType

FULL_CFG = dict(N=100000, E=400000, W=8, DIM=3, HID=32)


def _ceil(a, b):
    return -(-a // b) * b


def make_geom(N, W):
    NLOC = N // W
    NLOCP = _ceil(NLOC, 128)
    NP = W * NLOCP
    assert NP % 4 == 0
    QS = NP // 4          # table rows per src-quarter
    assert QS % 128 == 0
    NW = NLOCP // 128     # dst windows per core
    return NLOC, NLOCP, NP, QS, NW


def wrap_idx16(idx):
    """Edge i -> [i%16, i//16], tiled to 128 partitions (int16)."""
    a = np.asarray(idx, np.int16).reshape(-1, 16).T
    return np.tile(a, (8, 1))


def _layout(cnt_by_core_cell, n_cells, pad_to=512):
    """Shared cell layout: R[cell] = max count over cores, total padded."""
    R = cnt_by_core_cell.max(axis=0).astype(np.int64)   # [n_cells]
    tot = int(R.sum())
    R[n_cells - 1] += _ceil(tot, pad_to) - tot
    coff = np.concatenate([[0], np.cumsum(R)])
    EP = int(coff[-1])
    return R, coff[:-1], EP


def _pairs(cell_list, CH, NW):
    """Chunk-level one-hot build plan.

    Returns (nbuilds, chunk_builds, last_pair, win_pairs, touched) where
    chunk_builds[k] = [(bcol, ws, [(ww, sl), ...]), ...] groups the windows
    chunk k touches into runs of <=2 adjacent windows (one 256-wide one-hot
    build per run), and last_pair[ww] = (k, bcol, sl) identifies the final
    accumulation into window ww (for the matmul stop flag).
    """
    pairs = [[] for _ in range(CH)]
    for (ww, off, ln) in cell_list:
        if ln == 0:
            continue
        k0, k1 = off // 128, (off + ln - 1) // 128
        for k in range(k0, k1 + 1):
            pairs[k].append(ww)
    chunk_builds = []
    nbuilds = 0
    last_pair = {}
    win_pairs = [0] * NW
    for k in range(CH):
        ws_list = sorted(set(pairs[k]))
        builds = []
        i = 0
        while i < len(ws_list):
            if i + 1 < len(ws_list) and ws_list[i + 1] == ws_list[i] + 1:
                run = [ws_list[i], ws_list[i + 1]]
                i += 2
            else:
                run = [ws_list[i]]
                i += 1
            items = [(ww, sl) for sl, ww in enumerate(run)]
            builds.append((nbuilds, run[0], items))
            for (ww, sl) in items:
                last_pair[ww] = (k, nbuilds, sl)
                win_pairs[ww] += 1
            nbuilds += 1
        chunk_builds.append(builds)
    return nbuilds, chunk_builds, last_pair, win_pairs


def host_prep(x, edge_index, edge_attr, params, cfg):
    """Build per-core input arrays + shared structural metadata."""
    N, E, W, DIM, HID = cfg['N'], cfg['E'], cfg['W'], cfg['DIM'], cfg['HID']
    NLOC, NLOCP, NP, QS, NW = make_geom(N, W)

    src = np.asarray(edge_index[0], np.int64)
    dst = np.asarray(edge_index[1], np.int64)
    ea = np.asarray(edge_attr, np.float32)

    tr = (src // NLOC) * NLOCP + (src % NLOC)    # gather-table row
    core = dst // NLOC
    q = tr // QS
    dl = dst % NLOC                              # dst local id
    w = dl // 128                                # dst window

    x = np.asarray(x, np.float32)

    # x4 table rows: [x(3)|0]*4 per node (for host expansion + L2 analog)
    x4 = np.zeros((NP, 128), np.float32)
    rows = np.arange(NP)
    rc, rl = rows // NLOCP, rows % NLOCP
    valid = rl < NLOC
    nid = np.clip(rc * NLOC + rl, 0, N - 1)
    for d in range(4):
        x4[valid, 32 * d:32 * d + DIM] = x[nid[valid]]
    x4 = x4.astype(BF16)

    # ===== layer 1 layout: cells = dst windows ============================
    key1 = core * NW + w
    cnt1 = np.bincount(key1, minlength=W * NW).reshape(W, NW)
    R1, coff1, EP1 = _layout(cnt1, NW)
    CH1, TB1 = EP1 // 128, EP1 // 512
    cell_list1 = [(ww, int(coff1[ww]), int(R1[ww])) for ww in range(NW)]
    nb1, cb1, lp1, wp1 = _pairs(cell_list1, CH1, NW)

    order1 = np.lexsort((dl, w, core))
    o_core = core[order1]
    o_w = w[order1]
    o_tr = tr[order1]
    o_dl = dl[order1]
    o_ea = ea[order1]

    g1 = np.zeros((W, EP1, 128), BF16)           # host-expanded x4[src]
    ct1 = np.zeros((W, 4, EP1), np.float32)
    dlv1 = np.full((W, EP1), -10000.0, np.float32)

    ckey = o_core * NW + o_w
    gs = np.flatnonzero(np.r_[True, ckey[1:] != ckey[:-1]])
    ge = np.r_[gs[1:], len(ckey)]
    for a, b in zip(gs, ge):
        c = int(o_core[a]); ww = int(o_w[a])
        o = int(coff1[ww]); n = b - a
        g1[c, o:o + n] = x4[o_tr[a:b]]
        ct1[c, 0, o:o + n] = 1.0
        ct1[c, 1:4, o:o + n] = o_ea[a:b].T
        dlv1[c, o:o + n] = o_dl[a:b].astype(np.float32)

    g1 = np.ascontiguousarray(g1.transpose(0, 2, 1))     # [W, 128, EP1]
    # expand c rows 32x into the 128-wide kron layout (pure replication)
    ct1 = np.ascontiguousarray(
        np.repeat(ct1, 32, axis=1)).astype(BF16)         # [W, 128, EP1]

    dsf1 = np.zeros((W, 128, max(nb1, 1)), np.float32)
    for k in range(CH1):
        for (bcol, ws, items) in cb1[k]:
            dsf1[:, :, bcol] = dlv1[:, k * 128:(k + 1) * 128] - 128.0 * ws

    # ===== layer 2 layout: cells = (src quarter, dst window) ==============
    key2 = (core * 4 + q) * NW + w
    cnt2 = np.bincount(key2, minlength=W * 4 * NW).reshape(W, 4 * NW)
    # pad each quarter's edge total to a 512 multiple (extend last cell)
    R2 = cnt2.max(axis=0).astype(np.int64).reshape(4, NW)
    for qq in range(4):
        tot = int(R2[qq].sum())
        R2[qq, NW - 1] += _ceil(tot, 512) - tot
    qsz = R2.sum(axis=1)
    qoff = np.concatenate([[0], np.cumsum(qsz)])
    EP2 = int(qsz.sum())
    CH2 = EP2 // 128
    coff2 = np.zeros((4, NW), np.int64)
    run = 0
    cell_list2 = []
    for qq in range(4):
        for ww in range(NW):
            coff2[qq, ww] = run
            cell_list2.append((ww, run, int(R2[qq, ww])))
            run += int(R2[qq, ww])
    assert run == EP2
    nb2, cb2, lp2, wp2 = _pairs(cell_list2, CH2, NW)

    calls = []                                   # (q, edge_off, size)
    for qq in range(4):
        o = 0
        while o < qsz[qq]:
            s = min(2048, int(qsz[qq]) - o)
            calls.append((qq, int(qoff[qq]) + o, s))
            o += s

    order2 = np.lexsort((dl, w, q, core))
    s_core = core[order2]
    s_q = q[order2]
    s_w = w[order2]
    s_tr = tr[order2]
    s_dl = dl[order2]
    s_ea = ea[order2]

    gidx = np.zeros((W, EP2), np.int64)
    ct2 = np.zeros((W, 4, EP2), np.float32)
    dlv2 = np.full((W, EP2), -10000.0, np.float32)

    ckey2 = (s_core * 4 + s_q) * NW + s_w
    gs = np.flatnonzero(np.r_[True, ckey2[1:] != ckey2[:-1]])
    ge = np.r_[gs[1:], len(ckey2)]
    for a, b in zip(gs, ge):
        c = int(s_core[a]); qq = int(s_q[a]); ww = int(s_w[a])
        o = int(coff2[qq, ww]); n = b - a
        gidx[c, o:o + n] = s_tr[a:b] - qq * QS
        ct2[c, 0, o:o + n] = 1.0
        ct2[c, 1:4, o:o + n] = s_ea[a:b].T
        dlv2[c, o:o + n] = s_dl[a:b].astype(np.float32)

    gidx16 = np.stack([wrap_idx16(gidx[c]) for c in range(W)])
    ct2 = np.ascontiguousarray(
        np.repeat(ct2, 32, axis=1)).astype(BF16)         # [W, 128, EP2]

    dsf2 = np.zeros((W, 128, max(nb2, 1)), np.float32)
    for k in range(CH2):
        for (bcol, ws, items) in cb2[k]:
            dsf2[:, :, bcol] = dlv2[:, k * 128:(k + 1) * 128] - 128.0 * ws

    # x_augT packed (per core): window w at [32*(w%3):+4, (w//3)*128:+128]
    XCOLS = _ceil(NW, 3) // 3 * 128
    xaug = np.zeros((W, 128, XCOLS), np.float32)
    for c in range(W):
        xa = np.zeros((4, NLOCP), np.float32)
        xa[:DIM, :NLOC] = x[c * NLOC:(c + 1) * NLOC].T
        xa[3, :NLOC] = 1.0
        for ww in range(NW):
            xaug[c, 32 * (ww % 3):32 * (ww % 3) + 4,
                 (ww // 3) * 128:(ww // 3) * 128 + 128] = \
                xa[:, ww * 128:(ww + 1) * 128]
    xaug = xaug.astype(BF16)

    # weights
    def stack_w(Wn, bn, in_c):
        S = np.zeros((128, HID), np.float32)
        B = bn.reshape(in_c, HID)
        S[0:in_c] = B
        for d in range(3):
            S[32 * (d + 1):32 * (d + 1) + in_c] = Wn[d].reshape(in_c, HID)
        return S.astype(BF16)

    w1stack = stack_w(np.asarray(params['Wn1'], np.float32),
                      np.asarray(params['bn1'], np.float32), DIM)
    w2stack = stack_w(np.asarray(params['Wn2'], np.float32),
                      np.asarray(params['bn2'], np.float32), HID)
    root1a = np.concatenate([np.asarray(params['root1'], np.float32),
                             np.asarray(params['b1'], np.float32)[None]], 0)
    root1a_p = np.zeros((68, HID), np.float32)
    for g in range(3):
        root1a_p[32 * g:32 * g + DIM] = root1a[:DIM]
        root1a_p[32 * g + 3] = root1a[DIM]
    root1a_p = root1a_p.astype(BF16)
    root2a = np.concatenate([np.asarray(params['root2'], np.float32),
                             np.asarray(params['b2'], np.float32)[None]],
                            0).astype(BF16)
    wf1a = np.concatenate([np.asarray(params['Wf1'], np.float32),
                           np.asarray(params['bf1'], np.float32)[None]],
                          0).astype(BF16)
    wf2a = np.concatenate([np.asarray(params['Wf2'], np.float32),
                           np.asarray(params['bf2'], np.float32)[None]],
                          0).astype(BF16)
    iota = np.tile(np.arange(256, dtype=np.float32)[None, :],
                   (128, 1)).astype(BF16)
    ident = np.eye(128, dtype=np.float32)

    meta = dict(NLOC=NLOC, NLOCP=NLOCP, NP=NP, QS=QS, NW=NW,
                EP1=EP1, CH1=CH1, TB1=TB1, nb1=nb1, cb1=cb1, lp1=lp1, wp1=wp1,
                EP2=EP2, CH2=CH2, nb2=nb2, cb2=cb2, lp2=lp2, wp2=wp2,
                coff1=[int(v) for v in coff1], R1=[int(v) for v in R1],
                qoff2=[int(v) for v in qoff],
                XCOLS=XCOLS, calls=calls, W=W, HID=HID, DIM=DIM)

    shared = dict(w1stack=w1stack, w2stack=w2stack,
                  root1a=root1a_p, root2a=root2a, wf1a=wf1a, wf2a=wf2a,
                  iota=iota, ident=ident)
    in_maps = []
    for c in range(W):
        m = dict(shared)
        m['g1'] = g1[c]
        m['ct1'] = ct1[c]
        m['dsf1'] = dsf1[c]
        m['gidx'] = gidx16[c]
        m['ct2'] = ct2[c]
        m['dsf2'] = dsf2[c]
        m['xaug'] = xaug[c]
        in_maps.append(m)
    return in_maps, meta


def build_bass(meta):
    W, HID = meta['W'], meta['HID']
    NLOCP, NP, QS, NW = meta['NLOCP'], meta['NP'], meta['QS'], meta['NW']
    EP1, CH1, nb1 = meta['EP1'], meta['CH1'], meta['nb1']
    EP2, CH2, nb2 = meta['EP2'], meta['CH2'], meta['nb2']
    cb1, lp1, wp1 = meta['cb1'], meta['lp1'], meta['wp1']
    cb2, lp2, wp2 = meta['cb2'], meta['lp2'], meta['wp2']
    XCOLS, calls = meta['XCOLS'], meta['calls']
    coff1, R1, qoff2 = meta['coff1'], meta['R1'], meta['qoff2']
    RANKS_Q = QS // 128
    f32, bf16, i16 = mybir.dt.float32, mybir.dt.bfloat16, mybir.dt.int16

    nc = bacc.Bacc("TRN2", target_bir_lowering=False, debug=False,
                   num_devices=W, enable_asserts=False)

    # I/O ------------------------------------------------------------------
    g1_d = nc.dram_tensor("g1", [128, EP1], bf16, kind="ExternalInput")
    ct1_d = nc.dram_tensor("ct1", [128, EP1], bf16, kind="ExternalInput")
    dsf1_d = nc.dram_tensor("dsf1", [128, max(nb1, 1)], f32,
                            kind="ExternalInput")
    gidx_d = nc.dram_tensor("gidx", [128, EP2 // 16], i16,
                            kind="ExternalInput")
    ct2_d = nc.dram_tensor("ct2", [128, EP2], bf16, kind="ExternalInput")
    dsf2_d = nc.dram_tensor("dsf2", [128, max(nb2, 1)], f32,
                            kind="ExternalInput")
    xaug_d = nc.dram_tensor("xaug", [128, XCOLS], bf16,
                            kind="ExternalInput")
    w1_d = nc.dram_tensor("w1stack", [128, HID], bf16, kind="ExternalInput")
    w2_d = nc.dram_tensor("w2stack", [128, HID], bf16, kind="ExternalInput")
    r1_d = nc.dram_tensor("root1a", [68, HID], bf16,
                          kind="ExternalInput")
    r2_d = nc.dram_tensor("root2a", [33, HID], bf16, kind="ExternalInput")
    wf1_d = nc.dram_tensor("wf1a", [33, HID], bf16, kind="ExternalInput")
    wf2_d = nc.dram_tensor("wf2a", [33, 1], bf16, kind="ExternalInput")
    iota_d = nc.dram_tensor("iota", [128, 256], bf16, kind="ExternalInput")
    id_d = nc.dram_tensor("ident", [128, 128], f32, kind="ExternalInput")
    out_d = nc.dram_tensor("out", [1, NLOCP], f32, kind="ExternalOutput")

    cc_in = nc.dram_tensor("cc_in", [NLOCP, HID], bf16)
    cc_out = nc.dram_tensor("cc_out", [NP, HID], bf16, addr_space="Shared")

    ctx = ExitStack()
    with tile.TileContext(nc) as tc:
      with ctx:
        const = ctx.enter_context(tc.tile_pool(name="const", bufs=1))
        big = ctx.enter_context(tc.tile_pool(name="big", bufs=1))
        pipe = ctx.enter_context(tc.tile_pool(name="pipe", bufs=2))
        ohp = ctx.enter_context(tc.tile_pool(name="ohp", bufs=8))
        tabp = ctx.enter_context(tc.tile_pool(name="tabp", bufs=2))

        nc.gpsimd.load_library(library_config.mlp)

        # ---- constant loads ----
        def load(pool, dram, shape, dtype):
            t = pool.tile(shape, dtype, tag="c_" + dram.name)
            nc.sync.dma_start(out=t[:], in_=dram[:, :])
            return t

        dsf1_s = load(const, dsf1_d, [128, max(nb1, 1)], f32)
        gidx_s = load(const, gidx_d, [128, EP2 // 16], i16)
        dsf2_s = load(const, dsf2_d, [128, max(nb2, 1)], f32)
        xaug_s = load(const, xaug_d, [128, XCOLS], bf16)
        w1_s = load(const, w1_d, [128, HID], bf16)
        w2_s = load(const, w2_d, [128, HID], bf16)
        r1_s = load(const, r1_d, [68, HID], bf16)
        r2_s = load(const, r2_d, [33, HID], bf16)
        wf1_s = load(const, wf1_d, [33, HID], bf16)
        wf2_s = load(const, wf2_d, [33, 1], bf16)
        iota_s = load(const, iota_d, [128, 256], bf16)
        id_s = load(const, id_d, [128, 128], f32)

        CHMAX = max(CH1, CH2)
        msg_s = big.tile([128, CHMAX * 32], bf16)   # all msgs of one layer
        h1c_s = big.tile([128, NW * 32], bf16)      # compact local h1
        h1T_s = big.tile([33, NLOCP], bf16)         # h1^T augmented
        h2T_s = h1T_s                               # reused for h2^T (WAR-safe)
        nc.vector.memset(h1T_s[32:33, :], 1.0)

        def do_msg(ps1, t, Gt, Ct, b, wstack):
            """One 512-edge block of pass 1: messages into msg_s."""
            Ft = pipe.tile([128, 512], bf16, tag="F")
            nc.vector.tensor_tensor(Ft[:], Gt[:, 512 * b:512 * b + 512],
                                    Ct[:, 512 * b:512 * b + 512], ALU.mult)
            Mp = ps1.tile([128, 128], f32, tag="M")
            for j in range(4):
                nc.tensor.matmul(Mp[:, 32 * j:32 * j + 32],
                                 Ft[:, 128 * j:128 * j + 128],
                                 wstack[:], start=True, stop=True)
            nc.scalar.activation(msg_s[:, t * 128:(t + 1) * 128],
                                 Mp[:], AF.Copy)

        def do_agg(aggp, t, cbuilds, lastp, dsf_s):
            """Inline aggregation of block t's four chunks."""
            for kk in range(4):
                k = 4 * t + kk
                for (bcol, ws, items) in cbuilds[k]:
                    wd = 128 * len(items)
                    oh = ohp.tile([128, 256], bf16, tag="oh")
                    nc.vector.tensor_scalar(oh[:, 0:wd], iota_s[:, 0:wd],
                                            dsf_s[:, bcol:bcol + 1], 0.0,
                                            ALU.is_equal, ALU.bypass)
                    for (ww, sl) in items:
                        a = aggp[:, 32 * ww:32 * ww + 32]
                        nc.tensor.matmul(
                            a, oh[:, 128 * sl:128 * sl + 128],
                            msg_s[:, 32 * k:32 * k + 32], start=False,
                            stop=(lastp[ww] == (k, bcol, sl)),
                            skip_group_check=True)

        def roots(layer, aggp, winp):
            for ww in range(NW):
                if layer == 1:
                    g = 32 * (ww % 3)
                    lhs = xaug_s[g:g + 4,
                                 (ww // 3) * 128:(ww // 3) * 128 + 128]
                    rr = r1_s[g:g + 4, :]
                else:
                    lhs = h1T_s[:, ww * 128:(ww + 1) * 128]
                    rr = r2_s[:]
                a = aggp[:, 32 * ww:32 * ww + 32]
                nc.tensor.matmul(a, lhs, rr, start=False,
                                 stop=(winp[ww] == 0), skip_group_check=True)


        def make_bywin(cbuilds, CH, klo=0, khi=None):
            khi = CH if khi is None else khi
            bywin = [[] for _ in range(NW)]
            for k in range(klo, khi):
                for (bcol, ws, items) in cbuilds[k]:
                    for (ww, sl) in items:
                        bywin[ww].append((k, bcol, sl, 128 * len(items)))
            return bywin

        def emit_root(layer, aggp, ww, stop):
            if layer == 1:
                g = 32 * (ww % 3)
                lhs = xaug_s[g:g + 4,
                             (ww // 3) * 128:(ww // 3) * 128 + 128]
                rr = r1_s[g:g + 4, :]
            else:
                lhs = h1T_s[:, ww * 128:(ww + 1) * 128]
                rr = r2_s[:]
            a = aggp[:, 32 * ww:32 * ww + 32]
            nc.tensor.matmul(a, lhs, rr, start=False, stop=stop,
                             skip_group_check=True)

        def emit_win(aggp, ww, plist, dsf_s, lastp):
            a = aggp[:, 32 * ww:32 * ww + 32]
            for (k, bcol, sl, wd) in plist:
                oh = ohp.tile([128, 256], bf16, tag="oh")
                nc.vector.tensor_scalar(oh[:, 0:wd], iota_s[:, 0:wd],
                                        dsf_s[:, bcol:bcol + 1], 0.0,
                                        ALU.is_equal, ALU.bypass)
                nc.tensor.matmul(a, oh[:, 128 * sl:128 * sl + 128],
                                 msg_s[:, 32 * k:32 * k + 32],
                                 start=False,
                                 stop=(lastp[ww] == (k, bcol, sl)),
                                 skip_group_check=True)

        def tail_a(layer, aggp, ps):
            """relu + transpose into hT (and h1c on layer 1)."""
            hT = h1T_s if layer == 1 else h2T_s
            for w0 in range(0, NW, 4):
                nsub = min(4, NW - w0)
                trp = ps.tile([32, 512], f32, tag="tr")
                for i in range(nsub):
                    ww = w0 + i
                    a = aggp[:, 32 * ww:32 * ww + 32]
                    if layer == 1:
                        nc.scalar.activation(
                            h1c_s[:, 32 * ww:32 * ww + 32], a, AF.Relu)
                    hf = pipe.tile([128, 32], f32, tag="hf")
                    nc.scalar.activation(hf[:], a, AF.Relu)
                    nc.tensor.transpose(trp[:, 128 * i:128 * i + 128],
                                        hf[:], id_s[:])
                span = 128 * nsub
                nc.scalar.activation(hT[0:32, 128 * w0:128 * w0 + span],
                                     trp[:, 0:span], AF.Copy)

        def tail_fc():
            with tc.tile_pool(name="psf", bufs=2, space="PSUM") as psf:
                for w0 in range(0, NW, 4):
                    span = 128 * min(4, NW - w0)
                    f1 = psf.tile([32, 512], f32, tag="f1")
                    nc.tensor.matmul(f1[:, 0:span], wf1_s[:],
                                     h2T_s[:, 128 * w0:128 * w0 + span],
                                     start=True, stop=True)
                    h3t = pipe.tile([33, 512], bf16, tag="h3t")
                    nc.scalar.activation(h3t[0:32, 0:span],
                                         f1[:, 0:span], AF.Relu)
                    nc.vector.memset(h3t[32:33, 0:span], 1.0)
                    f2 = psf.tile([1, 512], f32, tag="f2")
                    nc.tensor.matmul(f2[:, 0:span], wf2_s[:],
                                     h3t[:, 0:span], start=True, stop=True)
                    ot = pipe.tile([1, 512], f32, tag="ot")
                    nc.scalar.activation(ot[:, 0:span], f2[:, 0:span],
                                         AF.Copy)
                    nc.sync.dma_start(
                        out=out_d[:, 128 * w0:128 * w0 + span],
                        in_=ot[:, 0:span])

        # ================= layer 1 =================
        with nc.named_scope("l1"), \
             tc.tile_pool(name="agg1", bufs=1, space="PSUM") as pa1:
            aggp = pa1.tile([128, NW * 32], f32, tag="agg")
            nc.vector.memset(aggp[:], 0.0)
            bywin1 = make_bywin(cb1, CH1)
            sched1 = {}
            for ww in range(NW):
                lb = (coff1[ww] + R1[ww] - 1) // 512
                sched1.setdefault(lb, []).append(ww)
            with tc.tile_pool(name="ps1", bufs=1, space="PSUM") as ps1:
                for eoff in range(0, EP1, 2048):
                    csz = min(2048, EP1 - eoff)
                    Gt = pipe.tile([128, 2048], bf16, tag="G")
                    nc.sync.dma_start(out=Gt[:, 0:csz],
                                      in_=g1_d[:, eoff:eoff + csz])
                    Ct = pipe.tile([128, 2048], bf16, tag="C4")
                    nc.sync.dma_start(out=Ct[:, 0:csz],
                                      in_=ct1_d[:, eoff:eoff + csz])
                    for b in range(csz // 512):
                        t = (eoff + b * 512) // 512
                        do_msg(ps1, t, Gt, Ct, b, w1_s)
                        for ww in sched1.get(t, []):
                            emit_root(1, aggp, ww, wp1[ww] == 0)
                            emit_win(aggp, ww, bywin1[ww], dsf1_s, lp1)
            with tc.tile_pool(name="pst1", bufs=1, space="PSUM") as pst:
                tail_a(1, aggp, pst)

        # ship compact h1, allgather
        nc.sync.dma_start(
            out=cc_in.ap().rearrange("(w p) h -> p w h", p=128),
            in_=h1c_s[:].rearrange("p (w h) -> p w h", h=HID))
        with nc.named_scope("allgather"):
            nc.gpsimd.collective_compute(
                "AllGather", ALU.bypass, replica_groups=[list(range(W))],
                ins=[cc_in.ap().opt()], outs=[cc_out.ap().opt()])

        # ================= layer 2 =================
        with nc.named_scope("l2"), \
             tc.tile_pool(name="agg2", bufs=1, space="PSUM") as pa2:
            aggp = pa2.tile([128, NW * 32], f32, tag="agg")
            nc.vector.memset(aggp[:], 0.0)
            bywin2q = [make_bywin(cb2, CH2, qoff2[q] // 128,
                                  qoff2[q + 1] // 128) for q in range(4)]
            def build_tq(q):
                """4x-replicated h1 table for quarter q, straight from DRAM."""
                tq = tabp.tile([128, RANKS_Q * 128], bf16, tag="tq")
                tq4 = tq[:].rearrange("p (r d h) -> p r d h", d=4, h=HID)
                srcv = cc_out.ap().rearrange("(q r p) h -> q p r h",
                                             q=4, p=128)[q]
                for d in range(4):
                    nc.sync.dma_start(out=tq4[:, :, d, :], in_=srcv)
                return tq

            with tc.tile_pool(name="ps1b", bufs=1, space="PSUM") as ps1b:
                tq = build_tq(0)
                for qq in range(4):
                    tq_next = build_tq(qq + 1) if qq < 3 else None
                    for (cq, eoff, csz) in calls:
                        if cq != qq:
                            continue
                        Gt = pipe.tile([128, 2048], bf16, tag="G")
                        g3 = Gt[:, 0:csz].rearrange("p (o n) -> p o n", o=1)
                        nc.gpsimd.dma_gather(
                            g3, tq[:],
                            gidx_s[:, eoff // 16:(eoff + csz) // 16],
                            csz, csz, 128, transpose=True,
                            single_packet=False,
                            sbuf_tokens_per_rank=128,
                            sbuf_free_dim_per_rank=256,
                            sbuf_free_dim_pad_per_rank=0, sbuf_byte_offset=0)
                        Ct = pipe.tile([128, 2048], bf16, tag="C4")
                        nc.sync.dma_start(out=Ct[:, 0:csz],
                                          in_=ct2_d[:, eoff:eoff + csz])
                        for b in range(csz // 512):
                            t = (eoff + b * 512) // 512
                            do_msg(ps1b, t, Gt, Ct, b, w2_s)
                    tq = tq_next
                    if qq < 3:
                        for ww in range(NW):
                            if qq == 0:
                                emit_root(2, aggp, ww, wp2[ww] == 0)
                            emit_win(aggp, ww, bywin2q[qq][ww], dsf2_s, lp2)
            # final quarter's aggregation fused with the relu/transpose tail
            with tc.tile_pool(name="pst2", bufs=1, space="PSUM") as pst:
                for w0 in range(0, NW, 4):
                    nsub = min(4, NW - w0)
                    for i in range(nsub):
                        emit_win(aggp, w0 + i, bywin2q[3][w0 + i],
                                 dsf2_s, lp2)
                    trp = pst.tile([32, 512], f32, tag="tr")
                    for i in range(nsub):
                        ww = w0 + i
                        a = aggp[:, 32 * ww:32 * ww + 32]
                        hf = pipe.tile([128, 32], f32, tag="hf")
                        nc.scalar.activation(hf[:], a, AF.Relu)
                        nc.tensor.transpose(trp[:, 128 * i:128 * i + 128],
                                            hf[:], id_s[:])
                    span = 128 * nsub
                    nc.scalar.activation(
                        h2T_s[0:32, 128 * w0:128 * w0 + span],
                        trp[:, 0:span], AF.Copy)
        tail_fc()
    return nc


def run_kernel(inputs, cfg=None, trace=False):
    cfg = cfg or FULL_CFG
    W = cfg['W']
    params = {k: inputs[k] for k in
              ('Wn1', 'bn1', 'root1', 'b1', 'Wn2', 'bn2', 'root2', 'b2',
               'Wf1', 'bf1', 'Wf2', 'bf2')}
    in_maps, meta = host_prep(inputs['x'], inputs['edge_index'],
                              inputs['edge_attr'], params, cfg)
    nc = build_bass(meta)
    nc.finalize()
    res = run_bass_kernel_spmd(nc, in_maps, core_ids=list(range(W)),
                               trace=trace)
    NLOC = meta['NLOC']
    out = np.zeros((cfg['N'], 1), np.float32)
    for c in range(W):
        out[c * NLOC:(c + 1) * NLOC, 0] = res.results[c]['out'][0, :NLOC]
    return out, res


def kernel(**inputs):
    out, _ = run_kernel(inputs)
    return out


# revision 43
# speedup vs baseline: 1.1502x; 1.1502x over previous
"""NNConv (gnn_message_passing) SPMD kernel for 8 trn2 NeuronCores.

Strategy (dst-sharded, both layers):
  - Each core owns a contiguous range of NLOC nodes (dst sharding). Edges are
    assigned to the core owning their dst.
  - msg = kron([1, ea], h_src) @ Wstack  (the NNConv per-edge weight matmul
    factorizes into a plain matmul over a 128-wide feature built from
    c = [1, ea0, ea1, ea2] outer h_src).
  - Layer 1 source features: host pre-expands x[src] into a [128, EP1] bf16
    stream (pure indexing/layout), streamed sequentially from DRAM — no
    device-side gather. L1 edges are sorted by (dst window, dst), minimizing
    aggregation one-hot pairs.
  - Layer 2 source features: DMA transpose-gather (256B rows, bf16,
    replicated 4x) from an SBUF-resident table built from the allgathered
    compact h1. L2 edges sorted by (src-quarter, dst window, dst).
  - Aggregation (segment sum over dst): one-hot PE matmuls into PSUM-resident
    per-window accumulators (window = 128 dst nodes), fused with the root-term
    matmul and ReLU.
  - One AllGather (compact h1, bf16) between the layers.
  - Edge layout is made identical across cores via shared R-tables
    (cell counts padded to the max over cores), so a single SPMD program
    works for all 8 cores.
"""

import sys

if '/opt/trn_rl_repo' not in sys.path:
    sys.path.insert(0, '/opt/trn_rl_repo')

from contextlib import ExitStack

import ml_dtypes
import numpy as np

import concourse.bacc as bacc
import concourse.bass as bass
from concourse import mybir, tile
from concourse.bass_utils import run_bass_kernel_spmd
from concourse import library_config

BF16 = ml_dtypes.bfloat16
AF = mybir.ActivationFunctionType
ALU = mybir.AluOpType

FULL_CFG = dict(N=100000, E=400000, W=8, DIM=3, HID=32)


def _ceil(a, b):
    return -(-a // b) * b


def make_geom(N, W):
    NLOC = N // W
    NLOCP = _ceil(NLOC, 128)
    NP = W * NLOCP
    assert NP % 4 == 0
    QS = NP // 4          # table rows per src-quarter
    assert QS % 128 == 0
    NW = NLOCP // 128     # dst windows per core
    return NLOC, NLOCP, NP, QS, NW


def wrap_idx16(idx):
    """Edge i -> [i%16, i//16], tiled to 128 partitions (int16)."""
    a = np.asarray(idx, np.int16).reshape(-1, 16).T
    return np.tile(a, (8, 1))


def _layout(cnt_by_core_cell, n_cells, pad_to=512):
    """Shared cell layout: R[cell] = max count over cores, total padded."""
    R = cnt_by_core_cell.max(axis=0).astype(np.int64)   # [n_cells]
    tot = int(R.sum())
    R[n_cells - 1] += _ceil(tot, pad_to) - tot
    coff = np.concatenate([[0], np.cumsum(R)])
    EP = int(coff[-1])
    return R, coff[:-1], EP


def _pairs(cell_list, CH, NW):
    """Chunk-level one-hot build plan.

    Returns (nbuilds, chunk_builds, last_pair, win_pairs, touched) where
    chunk_builds[k] = [(bcol, ws, [(ww, sl), ...]), ...] groups the windows
    chunk k touches into runs of <=2 adjacent windows (one 256-wide one-hot
    build per run), and last_pair[ww] = (k, bcol, sl) identifies the final
    accumulation into window ww (for the matmul stop flag).
    """
    pairs = [[] for _ in range(CH)]
    for (ww, off, ln) in cell_list:
        if ln == 0:
            continue
        k0, k1 = off // 128, (off + ln - 1) // 128
        for k in range(k0, k1 + 1):
            pairs[k].append(ww)
    chunk_builds = []
    nbuilds = 0
    last_pair = {}
    win_pairs = [0] * NW
    for k in range(CH):
        ws_list = sorted(set(pairs[k]))
        builds = []
        i = 0
        while i < len(ws_list):
            if i + 1 < len(ws_list) and ws_list[i + 1] == ws_list[i] + 1:
                run = [ws_list[i], ws_list[i + 1]]
                i += 2
            else:
                run = [ws_list[i]]
                i += 1
            items = [(ww, sl) for sl, ww in enumerate(run)]
            builds.append((nbuilds, run[0], items))
            for (ww, sl) in items:
                last_pair[ww] = (k, nbuilds, sl)
                win_pairs[ww] += 1
            nbuilds += 1
        chunk_builds.append(builds)
    return nbuilds, chunk_builds, last_pair, win_pairs


def host_prep(x, edge_index, edge_attr, params, cfg):
    """Build per-core input arrays + shared structural metadata."""
    N, E, W, DIM, HID = cfg['N'], cfg['E'], cfg['W'], cfg['DIM'], cfg['HID']
    NLOC, NLOCP, NP, QS, NW = make_geom(N, W)

    src = np.asarray(edge_index[0], np.int64)
    dst = np.asarray(edge_index[1], np.int64)
    ea = np.asarray(edge_attr, np.float32)

    tr = (src // NLOC) * NLOCP + (src % NLOC)    # gather-table row
    core = dst // NLOC
    q = tr // QS
    dl = dst % NLOC                              # dst local id
    w = dl // 128                                # dst window

    x = np.asarray(x, np.float32)

    # x4 table rows: [x(3)|0]*4 per node (for host expansion + L2 analog)
    x4 = np.zeros((NP, 128), np.float32)
    rows = np.arange(NP)
    rc, rl = rows // NLOCP, rows % NLOCP
    valid = rl < NLOC
    nid = np.clip(rc * NLOC + rl, 0, N - 1)
    for d in range(4):
        x4[valid, 32 * d:32 * d + DIM] = x[nid[valid]]
    x4 = x4.astype(BF16)

    # ===== layer 1 layout: cells = dst windows ============================
    key1 = core * NW + w
    cnt1 = np.bincount(key1, minlength=W * NW).reshape(W, NW)
    R1, coff1, EP1 = _layout(cnt1, NW)
    CH1, TB1 = EP1 // 128, EP1 // 512
    cell_list1 = [(ww, int(coff1[ww]), int(R1[ww])) for ww in range(NW)]
    nb1, cb1, lp1, wp1 = _pairs(cell_list1, CH1, NW)

    order1 = np.lexsort((dl, w, core))
    o_core = core[order1]
    o_w = w[order1]
    o_tr = tr[order1]
    o_dl = dl[order1]
    o_ea = ea[order1]

    g1 = np.zeros((W, EP1, 128), BF16)           # host-expanded x4[src]
    ct1 = np.zeros((W, 4, EP1), np.float32)
    dlv1 = np.full((W, EP1), -10000.0, np.float32)

    ckey = o_core * NW + o_w
    gs = np.flatnonzero(np.r_[True, ckey[1:] != ckey[:-1]])
    ge = np.r_[gs[1:], len(ckey)]
    for a, b in zip(gs, ge):
        c = int(o_core[a]); ww = int(o_w[a])
        o = int(coff1[ww]); n = b - a
        g1[c, o:o + n] = x4[o_tr[a:b]]
        ct1[c, 0, o:o + n] = 1.0
        ct1[c, 1:4, o:o + n] = o_ea[a:b].T
        dlv1[c, o:o + n] = o_dl[a:b].astype(np.float32)

    g1 = np.ascontiguousarray(g1.transpose(0, 2, 1))     # [W, 128, EP1]
    # expand c rows 32x into the 128-wide kron layout (pure replication)
    ct1 = np.ascontiguousarray(
        np.repeat(ct1, 32, axis=1)).astype(BF16)         # [W, 128, EP1]

    dsf1 = np.zeros((W, 128, max(nb1, 1)), np.float32)
    for k in range(CH1):
        for (bcol, ws, items) in cb1[k]:
            dsf1[:, :, bcol] = dlv1[:, k * 128:(k + 1) * 128] - 128.0 * ws

    # ===== layer 2 layout: cells = (src quarter, dst window) ==============
    key2 = (core * 4 + q) * NW + w
    cnt2 = np.bincount(key2, minlength=W * 4 * NW).reshape(W, 4 * NW)
    # pad each quarter's edge total to a 512 multiple (extend last cell)
    R2 = cnt2.max(axis=0).astype(np.int64).reshape(4, NW)
    for qq in range(4):
        tot = int(R2[qq].sum())
        R2[qq, NW - 1] += _ceil(tot, 512) - tot
    qsz = R2.sum(axis=1)
    qoff = np.concatenate([[0], np.cumsum(qsz)])
    EP2 = int(qsz.sum())
    CH2 = EP2 // 128
    coff2 = np.zeros((4, NW), np.int64)
    run = 0
    cell_list2 = []
    for qq in range(4):
        for ww in range(NW):
            coff2[qq, ww] = run
            cell_list2.append((ww, run, int(R2[qq, ww])))
            run += int(R2[qq, ww])
    assert run == EP2
    nb2, cb2, lp2, wp2 = _pairs(cell_list2, CH2, NW)

    calls = []                                   # (q, edge_off, size)
    for qq in range(4):
        o = 0
        while o < qsz[qq]:
            s = min(2048, int(qsz[qq]) - o)
            calls.append((qq, int(qoff[qq]) + o, s))
            o += s

    order2 = np.lexsort((dl, w, q, core))
    s_core = core[order2]
    s_q = q[order2]
    s_w = w[order2]
    s_tr = tr[order2]
    s_dl = dl[order2]
    s_ea = ea[order2]

    gidx = np.zeros((W, EP2), np.int64)
    ct2 = np.zeros((W, 4, EP2), np.float32)
    dlv2 = np.full((W, EP2), -10000.0, np.float32)

    ckey2 = (s_core * 4 + s_q) * NW + s_w
    gs = np.flatnonzero(np.r_[True, ckey2[1:] != ckey2[:-1]])
    ge = np.r_[gs[1:], len(ckey2)]
    for a, b in zip(gs, ge):
        c = int(s_core[a]); qq = int(s_q[a]); ww = int(s_w[a])
        o = int(coff2[qq, ww]); n = b - a
        gidx[c, o:o + n] = s_tr[a:b] - qq * QS
        ct2[c, 0, o:o + n] = 1.0
        ct2[c, 1:4, o:o + n] = s_ea[a:b].T
        dlv2[c, o:o + n] = s_dl[a:b].astype(np.float32)

    gidx16 = np.stack([wrap_idx16(gidx[c]) for c in range(W)])
    ct2 = np.ascontiguousarray(
        np.repeat(ct2, 32, axis=1)).astype(BF16)         # [W, 128, EP2]

    dsf2 = np.zeros((W, 128, max(nb2, 1)), np.float32)
    for k in range(CH2):
        for (bcol, ws, items) in cb2[k]:
            dsf2[:, :, bcol] = dlv2[:, k * 128:(k + 1) * 128] - 128.0 * ws

    # x_augT packed (per core): window w at [32*(w%3):+4, (w//3)*128:+128]
    XCOLS = _ceil(NW, 3) // 3 * 128
    xaug = np.zeros((W, 128, XCOLS), np.float32)
    for c in range(W):
        xa = np.zeros((4, NLOCP), np.float32)
        xa[:DIM, :NLOC] = x[c * NLOC:(c + 1) * NLOC].T
        xa[3, :NLOC] = 1.0
        for ww in range(NW):
            xaug[c, 32 * (ww % 3):32 * (ww % 3) + 4,
                 (ww // 3) * 128:(ww // 3) * 128 + 128] = \
                xa[:, ww * 128:(ww + 1) * 128]

    # weights
    def stack_w(Wn, bn, in_c):
        S = np.zeros((128, HID), np.float32)
        B = bn.reshape(in_c, HID)
        S[0:in_c] = B
        for d in range(3):
            S[32 * (d + 1):32 * (d + 1) + in_c] = Wn[d].reshape(in_c, HID)
        return S.astype(BF16)

    w1stack = stack_w(np.asarray(params['Wn1'], np.float32),
                      np.asarray(params['bn1'], np.float32), DIM)
    w2stack = stack_w(np.asarray(params['Wn2'], np.float32),
                      np.asarray(params['bn2'], np.float32), HID)
    root1a = np.concatenate([np.asarray(params['root1'], np.float32),
                             np.asarray(params['b1'], np.float32)[None]], 0)
    root1a_p = np.zeros((68, HID), np.float32)
    for g in range(3):
        root1a_p[32 * g:32 * g + DIM] = root1a[:DIM]
        root1a_p[32 * g + 3] = root1a[DIM]
    root2a = np.concatenate([np.asarray(params['root2'], np.float32),
                             np.asarray(params['b2'], np.float32)[None]],
                            0).astype(BF16)
    wf1a = np.concatenate([np.asarray(params['Wf1'], np.float32),
                           np.asarray(params['bf1'], np.float32)[None]],
                          0).astype(BF16)
    wf2a = np.concatenate([np.asarray(params['Wf2'], np.float32),
                           np.asarray(params['bf2'], np.float32)[None]],
                          0).astype(BF16)
    iota = np.tile(np.arange(256, dtype=np.float32)[None, :],
                   (128, 1)).astype(BF16)
    ident = np.eye(128, dtype=np.float32)

    meta = dict(NLOC=NLOC, NLOCP=NLOCP, NP=NP, QS=QS, NW=NW,
                EP1=EP1, CH1=CH1, TB1=TB1, nb1=nb1, cb1=cb1, lp1=lp1, wp1=wp1,
                EP2=EP2, CH2=CH2, nb2=nb2, cb2=cb2, lp2=lp2, wp2=wp2,
                coff1=[int(v) for v in coff1], R1=[int(v) for v in R1],
                qoff2=[int(v) for v in qoff],
                XCOLS=XCOLS, calls=calls, W=W, HID=HID, DIM=DIM)

    shared = dict(w1stack=w1stack, w2stack=w2stack,
                  root1a=root1a_p, root2a=root2a, wf1a=wf1a, wf2a=wf2a,
                  iota=iota, ident=ident)
    in_maps = []
    for c in range(W):
        m = dict(shared)
        m['g1'] = g1[c]
        m['ct1'] = ct1[c]
        m['dsf1'] = dsf1[c]
        m['gidx'] = gidx16[c]
        m['ct2'] = ct2[c]
        m['dsf2'] = dsf2[c]
        m['xaug'] = xaug[c]
        in_maps.append(m)
    return in_maps, meta


def build_bass(meta):
    W, HID = meta['W'], meta['HID']
    NLOCP, NP, QS, NW = meta['NLOCP'], meta['NP'], meta['QS'], meta['NW']
    EP1, CH1, nb1 = meta['EP1'], meta['CH1'], meta['nb1']
    EP2, CH2, nb2 = meta['EP2'], meta['CH2'], meta['nb2']
    cb1, lp1, wp1 = meta['cb1'], meta['lp1'], meta['wp1']
    cb2, lp2, wp2 = meta['cb2'], meta['lp2'], meta['wp2']
    XCOLS, calls = meta['XCOLS'], meta['calls']
    coff1, R1, qoff2 = meta['coff1'], meta['R1'], meta['qoff2']
    RANKS_Q = QS // 128
    f32, bf16, i16 = mybir.dt.float32, mybir.dt.bfloat16, mybir.dt.int16

    nc = bacc.Bacc("TRN2", target_bir_lowering=False, debug=False,
                   num_devices=W, enable_asserts=False)

    # I/O ------------------------------------------------------------------
    g1_d = nc.dram_tensor("g1", [128, EP1], bf16, kind="ExternalInput")
    ct1_d = nc.dram_tensor("ct1", [128, EP1], bf16, kind="ExternalInput")
    dsf1_d = nc.dram_tensor("dsf1", [128, max(nb1, 1)], f32,
                            kind="ExternalInput")
    gidx_d = nc.dram_tensor("gidx", [128, EP2 // 16], i16,
                            kind="ExternalInput")
    ct2_d = nc.dram_tensor("ct2", [128, EP2], bf16, kind="ExternalInput")
    dsf2_d = nc.dram_tensor("dsf2", [128, max(nb2, 1)], f32,
                            kind="ExternalInput")
    xaug_d = nc.dram_tensor("xaug", [128, XCOLS], f32, kind="ExternalInput")
    w1_d = nc.dram_tensor("w1stack", [128, HID], bf16, kind="ExternalInput")
    w2_d = nc.dram_tensor("w2stack", [128, HID], bf16, kind="ExternalInput")
    r1_d = nc.dram_tensor("root1a", [68, HID], f32, kind="ExternalInput")
    r2_d = nc.dram_tensor("root2a", [33, HID], bf16, kind="ExternalInput")
    wf1_d = nc.dram_tensor("wf1a", [33, HID], bf16, kind="ExternalInput")
    wf2_d = nc.dram_tensor("wf2a", [33, 1], bf16, kind="ExternalInput")
    iota_d = nc.dram_tensor("iota", [128, 256], bf16, kind="ExternalInput")
    id_d = nc.dram_tensor("ident", [128, 128], f32, kind="ExternalInput")
    out_d = nc.dram_tensor("out", [1, NLOCP], f32, kind="ExternalOutput")

    cc_in = nc.dram_tensor("cc_in", [NLOCP, HID], bf16)
    cc_out = nc.dram_tensor("cc_out", [NP, HID], bf16, addr_space="Shared")

    ctx = ExitStack()
    with tile.TileContext(nc) as tc:
      with ctx:
        const = ctx.enter_context(tc.tile_pool(name="const", bufs=1))
        big = ctx.enter_context(tc.tile_pool(name="big", bufs=1))
        pipe = ctx.enter_context(tc.tile_pool(name="pipe", bufs=2))
        ohp = ctx.enter_context(tc.tile_pool(name="ohp", bufs=8))
        tabp = ctx.enter_context(tc.tile_pool(name="tabp", bufs=1))

        nc.gpsimd.load_library(library_config.mlp)

        # ---- constant loads ----
        def load(pool, dram, shape, dtype):
            t = pool.tile(shape, dtype, tag="c_" + dram.name)
            nc.sync.dma_start(out=t[:], in_=dram[:, :])
            return t

        dsf1_s = load(const, dsf1_d, [128, max(nb1, 1)], f32)
        gidx_s = load(const, gidx_d, [128, EP2 // 16], i16)
        dsf2_s = load(const, dsf2_d, [128, max(nb2, 1)], f32)
        xaug_s = load(const, xaug_d, [128, XCOLS], f32)
        w1_s = load(const, w1_d, [128, HID], bf16)
        w2_s = load(const, w2_d, [128, HID], bf16)
        r1_s = load(const, r1_d, [68, HID], f32)
        r2_s = load(const, r2_d, [33, HID], bf16)
        wf1_s = load(const, wf1_d, [33, HID], bf16)
        wf2_s = load(const, wf2_d, [33, 1], bf16)
        iota_s = load(const, iota_d, [128, 256], bf16)
        id_s = load(const, id_d, [128, 128], f32)

        CHMAX = max(CH1, CH2)
        msg_s = big.tile([128, CHMAX * 32], bf16)   # all msgs of one layer
        h1c_s = big.tile([128, NW * 32], bf16)      # compact local h1
        h1T_s = big.tile([33, NLOCP], bf16)         # h1^T augmented
        h2T_s = h1T_s                               # reused for h2^T (WAR-safe)
        nc.vector.memset(h1T_s[32:33, :], 1.0)

        def do_msg(ps1, t, Gt, Ct, b, wstack):
            """One 512-edge block of pass 1: messages into msg_s."""
            Ft = pipe.tile([128, 512], bf16, tag="F")
            nc.vector.tensor_tensor(Ft[:], Gt[:, 512 * b:512 * b + 512],
                                    Ct[:, 512 * b:512 * b + 512], ALU.mult)
            Mp = ps1.tile([128, 128], f32, tag="M")
            for j in range(4):
                nc.tensor.matmul(Mp[:, 32 * j:32 * j + 32],
                                 Ft[:, 128 * j:128 * j + 128],
                                 wstack[:], start=True, stop=True)
            nc.scalar.activation(msg_s[:, t * 128:(t + 1) * 128],
                                 Mp[:], AF.Copy)

        def do_agg(aggp, t, cbuilds, lastp, dsf_s):
            """Inline aggregation of block t's four chunks."""
            for kk in range(4):
                k = 4 * t + kk
                for (bcol, ws, items) in cbuilds[k]:
                    wd = 128 * len(items)
                    oh = ohp.tile([128, 256], bf16, tag="oh")
                    nc.vector.tensor_scalar(oh[:, 0:wd], iota_s[:, 0:wd],
                                            dsf_s[:, bcol:bcol + 1], 0.0,
                                            ALU.is_equal, ALU.bypass)
                    for (ww, sl) in items:
                        a = aggp[:, 32 * ww:32 * ww + 32]
                        nc.tensor.matmul(
                            a, oh[:, 128 * sl:128 * sl + 128],
                            msg_s[:, 32 * k:32 * k + 32], start=False,
                            stop=(lastp[ww] == (k, bcol, sl)),
                            skip_group_check=True)

        def roots(layer, aggp, winp):
            for ww in range(NW):
                if layer == 1:
                    g = 32 * (ww % 3)
                    lhs = xaug_s[g:g + 4,
                                 (ww // 3) * 128:(ww // 3) * 128 + 128]
                    rr = r1_s[g:g + 4, :]
                else:
                    lhs = h1T_s[:, ww * 128:(ww + 1) * 128]
                    rr = r2_s[:]
                a = aggp[:, 32 * ww:32 * ww + 32]
                nc.tensor.matmul(a, lhs, rr, start=False,
                                 stop=(winp[ww] == 0), skip_group_check=True)


        def make_bywin(cbuilds, CH, klo=0, khi=None):
            khi = CH if khi is None else khi
            bywin = [[] for _ in range(NW)]
            for k in range(klo, khi):
                for (bcol, ws, items) in cbuilds[k]:
                    for (ww, sl) in items:
                        bywin[ww].append((k, bcol, sl, 128 * len(items)))
            return bywin

        def emit_root(layer, aggp, ww, stop):
            if layer == 1:
                g = 32 * (ww % 3)
                lhs = xaug_s[g:g + 4,
                             (ww // 3) * 128:(ww // 3) * 128 + 128]
                rr = r1_s[g:g + 4, :]
            else:
                lhs = h1T_s[:, ww * 128:(ww + 1) * 128]
                rr = r2_s[:]
            a = aggp[:, 32 * ww:32 * ww + 32]
            nc.tensor.matmul(a, lhs, rr, start=False, stop=stop,
                             skip_group_check=True)

        def emit_win(aggp, ww, plist, dsf_s, lastp):
            a = aggp[:, 32 * ww:32 * ww + 32]
            for (k, bcol, sl, wd) in plist:
                oh = ohp.tile([128, 256], bf16, tag="oh")
                nc.vector.tensor_scalar(oh[:, 0:wd], iota_s[:, 0:wd],
                                        dsf_s[:, bcol:bcol + 1], 0.0,
                                        ALU.is_equal, ALU.bypass)
                nc.tensor.matmul(a, oh[:, 128 * sl:128 * sl + 128],
                                 msg_s[:, 32 * k:32 * k + 32],
                                 start=False,
                                 stop=(lastp[ww] == (k, bcol, sl)),
                                 skip_group_check=True)

        def tail_a(layer, aggp, ps):
            """relu + transpose into hT (and h1c on layer 1)."""
            hT = h1T_s if layer == 1 else h2T_s
            for w0 in range(0, NW, 4):
                nsub = min(4, NW - w0)
                trp = ps.tile([32, 512], f32, tag="tr")
                for i in range(nsub):
                    ww = w0 + i
                    a = aggp[:, 32 * ww:32 * ww + 32]
                    if layer == 1:
                        nc.scalar.activation(
                            h1c_s[:, 32 * ww:32 * ww + 32], a, AF.Relu)
                    hf = pipe.tile([128, 32], f32, tag="hf")
                    nc.scalar.activation(hf[:], a, AF.Relu)
                    nc.tensor.transpose(trp[:, 128 * i:128 * i + 128],
                                        hf[:], id_s[:])
                span = 128 * nsub
                nc.scalar.activation(hT[0:32, 128 * w0:128 * w0 + span],
                                     trp[:, 0:span], AF.Copy)

        def tail_fc():
            with tc.tile_pool(name="psf", bufs=2, space="PSUM") as psf:
                for w0 in range(0, NW, 4):
                    span = 128 * min(4, NW - w0)
                    f1 = psf.tile([32, 512], f32, tag="f1")
                    nc.tensor.matmul(f1[:, 0:span], wf1_s[:],
                                     h2T_s[:, 128 * w0:128 * w0 + span],
                                     start=True, stop=True)
                    h3t = pipe.tile([33, 512], bf16, tag="h3t")
                    nc.scalar.activation(h3t[0:32, 0:span],
                                         f1[:, 0:span], AF.Relu)
                    nc.vector.memset(h3t[32:33, 0:span], 1.0)
                    f2 = psf.tile([1, 512], f32, tag="f2")
                    nc.tensor.matmul(f2[:, 0:span], wf2_s[:],
                                     h3t[:, 0:span], start=True, stop=True)
                    ot = pipe.tile([1, 512], f32, tag="ot")
                    nc.scalar.activation(ot[:, 0:span], f2[:, 0:span],
                                         AF.Copy)
                    nc.sync.dma_start(
                        out=out_d[:, 128 * w0:128 * w0 + span],
                        in_=ot[:, 0:span])

        # ================= layer 1 =================
        with nc.named_scope("l1"), \
             tc.tile_pool(name="agg1", bufs=1, space="PSUM") as pa1:
            aggp = pa1.tile([128, NW * 32], f32, tag="agg")
            nc.vector.memset(aggp[:], 0.0)
            bywin1 = make_bywin(cb1, CH1)
            sched1 = {}
            for ww in range(NW):
                lb = (coff1[ww] + R1[ww] - 1) // 512
                sched1.setdefault(lb, []).append(ww)
            with tc.tile_pool(name="ps1", bufs=1, space="PSUM") as ps1:
                for eoff in range(0, EP1, 2048):
                    csz = min(2048, EP1 - eoff)
                    Gt = pipe.tile([128, 2048], bf16, tag="G")
                    nc.sync.dma_start(out=Gt[:, 0:csz],
                                      in_=g1_d[:, eoff:eoff + csz])
                    Ct = pipe.tile([128, 2048], bf16, tag="C4")
                    nc.sync.dma_start(out=Ct[:, 0:csz],
                                      in_=ct1_d[:, eoff:eoff + csz])
                    for b in range(csz // 512):
                        t = (eoff + b * 512) // 512
                        do_msg(ps1, t, Gt, Ct, b, w1_s)
                        for ww in sched1.get(t, []):
                            emit_root(1, aggp, ww, wp1[ww] == 0)
                            emit_win(aggp, ww, bywin1[ww], dsf1_s, lp1)
            with tc.tile_pool(name="pst1", bufs=1, space="PSUM") as pst:
                tail_a(1, aggp, pst)

        # ship compact h1, allgather
        nc.sync.dma_start(
            out=cc_in.ap().rearrange("(w p) h -> p w h", p=128),
            in_=h1c_s[:].rearrange("p (w h) -> p w h", h=HID))
        with nc.named_scope("allgather"):
            nc.gpsimd.collective_compute(
                "AllGather", ALU.bypass, replica_groups=[list(range(W))],
                ins=[cc_in.ap().opt()], outs=[cc_out.ap().opt()])

        # ================= layer 2 =================
        with nc.named_scope("l2"), \
             tc.tile_pool(name="agg2", bufs=1, space="PSUM") as pa2:
            aggp = pa2.tile([128, NW * 32], f32, tag="agg")
            nc.vector.memset(aggp[:], 0.0)
            bywin2q = [make_bywin(cb2, CH2, qoff2[q] // 128,
                                  qoff2[q + 1] // 128) for q in range(4)]
            with tc.tile_pool(name="ps1b", bufs=1, space="PSUM") as ps1b:
                for qq in range(4):
                    cw = tabp.tile([128, RANKS_Q * 32], bf16, tag="cw")
                    nc.sync.dma_start(
                        out=cw[:].rearrange("p (r h) -> p r h", h=HID),
                        in_=cc_out.ap().rearrange("(q r p) h -> q p r h",
                                                  q=4, p=128)[qq])
                    tq = tabp.tile([128, RANKS_Q * 128], bf16, tag="tq")
                    tq4 = tq[:].rearrange("p (r d h) -> p r d h", d=4, h=HID)
                    cw3 = cw[:].rearrange("p (r h) -> p r h", h=HID)
                    for d in range(4):
                        nc.vector.tensor_copy(tq4[:, :, d, :], cw3)
                    for (cq, eoff, csz) in calls:
                        if cq != qq:
                            continue
                        Gt = pipe.tile([128, 2048], bf16, tag="G")
                        g3 = Gt[:, 0:csz].rearrange("p (o n) -> p o n", o=1)
                        nc.gpsimd.dma_gather(
                            g3, tq[:],
                            gidx_s[:, eoff // 16:(eoff + csz) // 16],
                            csz, csz, 128, transpose=True,
                            single_packet=False,
                            sbuf_tokens_per_rank=128,
                            sbuf_free_dim_per_rank=256,
                            sbuf_free_dim_pad_per_rank=0, sbuf_byte_offset=0)
                        Ct = pipe.tile([128, 2048], bf16, tag="C4")
                        nc.sync.dma_start(out=Ct[:, 0:csz],
                                          in_=ct2_d[:, eoff:eoff + csz])
                        for b in range(csz // 512):
                            t = (eoff + b * 512) // 512
                            do_msg(ps1b, t, Gt, Ct, b, w2_s)
                    if qq < 3:
                        for ww in range(NW):
                            if qq == 0:
                                emit_root(2, aggp, ww, wp2[ww] == 0)
                            emit_win(aggp, ww, bywin2q[qq][ww], dsf2_s, lp2)
            # final quarter's aggregation fused with the relu/transpose tail
            with tc.tile_pool(name="pst2", bufs=1, space="PSUM") as pst:
                for w0 in range(0, NW, 4):
                    nsub = min(4, NW - w0)
                    for i in range(nsub):
                        emit_win(aggp, w0 + i, bywin2q[3][w0 + i],
                                 dsf2_s, lp2)
                    trp = pst.tile([32, 512], f32, tag="tr")
                    for i in range(nsub):
                        ww = w0 + i
                        a = aggp[:, 32 * ww:32 * ww + 32]
                        hf = pipe.tile([128, 32], f32, tag="hf")
                        nc.scalar.activation(hf[:], a, AF.Relu)
                        nc.tensor.transpose(trp[:, 128 * i:128 * i + 128],
                                            hf[:], id_s[:])
                    span = 128 * nsub
                    nc.scalar.activation(
                        h2T_s[0:32, 128 * w0:128 * w0 + span],
                        trp[:, 0:span], AF.Copy)
        tail_fc()
    return nc


def run_kernel(inputs, cfg=None, trace=False):
    cfg = cfg or FULL_CFG
    W = cfg['W']
    params = {k: inputs[k] for k in
              ('Wn1', 'bn1', 'root1', 'b1', 'Wn2', 'bn2', 'root2', 'b2',
               'Wf1', 'bf1', 'Wf2', 'bf2')}
    in_maps, meta = host_prep(inputs['x'], inputs['edge_index'],
                              inputs['edge_attr'], params, cfg)
    nc = build_bass(meta)
    nc.finalize()
    res = run_bass_kernel_spmd(nc, in_maps, core_ids=list(range(W)),
                               trace=trace)
    NLOC = meta['NLOC']
    out = np.zeros((cfg['N'], 1), np.float32)
    for c in range(W):
        out[c * NLOC:(c + 1) * NLOC, 0] = res.results[c]['out'][0, :NLOC]
    return out, res


def kernel(**inputs):
    out, _ = run_kernel(inputs)
    return out


# revision 44
# speedup vs baseline: 1.2218x; 1.0623x over previous
"""NNConv (gnn_message_passing) SPMD kernel for 8 trn2 NeuronCores.

Strategy (dst-sharded, both layers):
  - Each core owns a contiguous range of NLOC nodes (dst sharding). Edges are
    assigned to the core owning their dst.
  - msg = kron([1, ea], h_src) @ Wstack  (the NNConv per-edge weight matmul
    factorizes into a plain matmul over a 128-wide feature built from
    c = [1, ea0, ea1, ea2] outer h_src).
  - Layer 1 source features: host pre-expands x[src] into a [128, EP1] bf16
    stream (pure indexing/layout), streamed sequentially from DRAM — no
    device-side gather. L1 edges are sorted by (dst window, dst), minimizing
    aggregation one-hot pairs.
  - Layer 2 source features: DMA transpose-gather (256B rows, bf16,
    replicated 4x) from an SBUF-resident table built from the allgathered
    compact h1. L2 edges sorted by (src-quarter, dst window, dst).
  - Aggregation (segment sum over dst): one-hot PE matmuls into PSUM-resident
    per-window accumulators (window = 128 dst nodes), fused with the root-term
    matmul and ReLU.
  - One AllGather (compact h1, bf16) between the layers.
  - Edge layout is made identical across cores via shared R-tables
    (cell counts padded to the max over cores), so a single SPMD program
    works for all 8 cores.
"""

import sys

if '/opt/trn_rl_repo' not in sys.path:
    sys.path.insert(0, '/opt/trn_rl_repo')

from contextlib import ExitStack

import ml_dtypes
import numpy as np

import concourse.bacc as bacc
import concourse.bass as bass
from concourse import mybir, tile
from concourse.bass_utils import run_bass_kernel_spmd
from concourse import library_config

BF16 = ml_dtypes.bfloat16
AF = mybir.ActivationFunctionType
ALU = mybir.AluOpType

FULL_CFG = dict(N=100000, E=400000, W=8, DIM=3, HID=32)


def _ceil(a, b):
    return -(-a // b) * b


def make_geom(N, W):
    NLOC = N // W
    NLOCP = _ceil(NLOC, 128)
    NP = W * NLOCP
    assert NP % 4 == 0
    QS = NP // 4          # table rows per src-quarter
    assert QS % 128 == 0
    NW = NLOCP // 128     # dst windows per core
    return NLOC, NLOCP, NP, QS, NW


def wrap_idx16(idx):
    """Edge i -> [i%16, i//16], tiled to 128 partitions (int16)."""
    a = np.asarray(idx, np.int16).reshape(-1, 16).T
    return np.tile(a, (8, 1))


def _layout(cnt_by_core_cell, n_cells, pad_to=512):
    """Shared cell layout: R[cell] = max count over cores, total padded."""
    R = cnt_by_core_cell.max(axis=0).astype(np.int64)   # [n_cells]
    tot = int(R.sum())
    R[n_cells - 1] += _ceil(tot, pad_to) - tot
    coff = np.concatenate([[0], np.cumsum(R)])
    EP = int(coff[-1])
    return R, coff[:-1], EP


def _pairs(cell_list, CH, NW):
    """Chunk-level one-hot build plan.

    Returns (nbuilds, chunk_builds, last_pair, win_pairs, touched) where
    chunk_builds[k] = [(bcol, ws, [(ww, sl), ...]), ...] groups the windows
    chunk k touches into runs of <=2 adjacent windows (one 256-wide one-hot
    build per run), and last_pair[ww] = (k, bcol, sl) identifies the final
    accumulation into window ww (for the matmul stop flag).
    """
    pairs = [[] for _ in range(CH)]
    for (ww, off, ln) in cell_list:
        if ln == 0:
            continue
        k0, k1 = off // 128, (off + ln - 1) // 128
        for k in range(k0, k1 + 1):
            pairs[k].append(ww)
    chunk_builds = []
    nbuilds = 0
    last_pair = {}
    win_pairs = [0] * NW
    for k in range(CH):
        ws_list = sorted(set(pairs[k]))
        builds = []
        i = 0
        while i < len(ws_list):
            if i + 1 < len(ws_list) and ws_list[i + 1] == ws_list[i] + 1:
                run = [ws_list[i], ws_list[i + 1]]
                i += 2
            else:
                run = [ws_list[i]]
                i += 1
            items = [(ww, sl) for sl, ww in enumerate(run)]
            builds.append((nbuilds, run[0], items))
            for (ww, sl) in items:
                last_pair[ww] = (k, nbuilds, sl)
                win_pairs[ww] += 1
            nbuilds += 1
        chunk_builds.append(builds)
    return nbuilds, chunk_builds, last_pair, win_pairs


def host_prep(x, edge_index, edge_attr, params, cfg):
    """Build per-core input arrays + shared structural metadata."""
    N, E, W, DIM, HID = cfg['N'], cfg['E'], cfg['W'], cfg['DIM'], cfg['HID']
    NLOC, NLOCP, NP, QS, NW = make_geom(N, W)

    src = np.asarray(edge_index[0], np.int64)
    dst = np.asarray(edge_index[1], np.int64)
    ea = np.asarray(edge_attr, np.float32)

    tr = (src // NLOC) * NLOCP + (src % NLOC)    # gather-table row
    core = dst // NLOC
    q = tr // QS
    dl = dst % NLOC                              # dst local id
    w = dl // 128                                # dst window

    x = np.asarray(x, np.float32)

    # x4 table rows: [x(3)|0]*4 per node (for host expansion + L2 analog)
    x4 = np.zeros((NP, 128), np.float32)
    rows = np.arange(NP)
    rc, rl = rows // NLOCP, rows % NLOCP
    valid = rl < NLOC
    nid = np.clip(rc * NLOC + rl, 0, N - 1)
    for d in range(4):
        x4[valid, 32 * d:32 * d + DIM] = x[nid[valid]]
    x4 = x4.astype(BF16)

    # ===== layer 1 layout: cells = dst windows ============================
    key1 = core * NW + w
    cnt1 = np.bincount(key1, minlength=W * NW).reshape(W, NW)
    R1, coff1, EP1 = _layout(cnt1, NW)
    CH1, TB1 = EP1 // 128, EP1 // 512
    cell_list1 = [(ww, int(coff1[ww]), int(R1[ww])) for ww in range(NW)]
    nb1, cb1, lp1, wp1 = _pairs(cell_list1, CH1, NW)

    order1 = np.lexsort((dl, w, core))
    o_core = core[order1]
    o_w = w[order1]
    o_tr = tr[order1]
    o_dl = dl[order1]
    o_ea = ea[order1]

    g1 = np.zeros((W, EP1, 128), BF16)           # host-expanded x4[src]
    ct1 = np.zeros((W, 4, EP1), np.float32)
    dlv1 = np.full((W, EP1), -10000.0, np.float32)

    ckey = o_core * NW + o_w
    gs = np.flatnonzero(np.r_[True, ckey[1:] != ckey[:-1]])
    ge = np.r_[gs[1:], len(ckey)]
    for a, b in zip(gs, ge):
        c = int(o_core[a]); ww = int(o_w[a])
        o = int(coff1[ww]); n = b - a
        g1[c, o:o + n] = x4[o_tr[a:b]]
        ct1[c, 0, o:o + n] = 1.0
        ct1[c, 1:4, o:o + n] = o_ea[a:b].T
        dlv1[c, o:o + n] = o_dl[a:b].astype(np.float32)

    g1 = np.ascontiguousarray(g1.transpose(0, 2, 1))     # [W, 128, EP1]
    # expand c rows 32x into the 128-wide kron layout (pure replication)
    ct1 = np.ascontiguousarray(
        np.repeat(ct1, 32, axis=1)).astype(BF16)         # [W, 128, EP1]

    dsf1 = np.zeros((W, 128, max(nb1, 1)), np.float32)
    for k in range(CH1):
        for (bcol, ws, items) in cb1[k]:
            dsf1[:, :, bcol] = dlv1[:, k * 128:(k + 1) * 128] - 128.0 * ws

    # ===== layer 2 layout: cells = (src quarter, dst window) ==============
    key2 = (core * 4 + q) * NW + w
    cnt2 = np.bincount(key2, minlength=W * 4 * NW).reshape(W, 4 * NW)
    # pad each quarter's edge total to a 512 multiple (extend last cell)
    R2 = cnt2.max(axis=0).astype(np.int64).reshape(4, NW)
    for qq in range(4):
        tot = int(R2[qq].sum())
        R2[qq, NW - 1] += _ceil(tot, 512) - tot
    qsz = R2.sum(axis=1)
    qoff = np.concatenate([[0], np.cumsum(qsz)])
    EP2 = int(qsz.sum())
    CH2 = EP2 // 128
    coff2 = np.zeros((4, NW), np.int64)
    run = 0
    cell_list2 = []
    for qq in range(4):
        for ww in range(NW):
            coff2[qq, ww] = run
            cell_list2.append((ww, run, int(R2[qq, ww])))
            run += int(R2[qq, ww])
    assert run == EP2
    nb2, cb2, lp2, wp2 = _pairs(cell_list2, CH2, NW)

    calls = []                                   # (q, edge_off, size)
    for qq in range(4):
        o = 0
        while o < qsz[qq]:
            s = min(2048, int(qsz[qq]) - o)
            calls.append((qq, int(qoff[qq]) + o, s))
            o += s

    order2 = np.lexsort((dl, w, q, core))
    s_core = core[order2]
    s_q = q[order2]
    s_w = w[order2]
    s_tr = tr[order2]
    s_dl = dl[order2]
    s_ea = ea[order2]

    gidx = np.zeros((W, EP2), np.int64)
    ct2 = np.zeros((W, 4, EP2), np.float32)
    dlv2 = np.full((W, EP2), -10000.0, np.float32)

    ckey2 = (s_core * 4 + s_q) * NW + s_w
    gs = np.flatnonzero(np.r_[True, ckey2[1:] != ckey2[:-1]])
    ge = np.r_[gs[1:], len(ckey2)]
    for a, b in zip(gs, ge):
        c = int(s_core[a]); qq = int(s_q[a]); ww = int(s_w[a])
        o = int(coff2[qq, ww]); n = b - a
        gidx[c, o:o + n] = s_tr[a:b] - qq * QS
        ct2[c, 0, o:o + n] = 1.0
        ct2[c, 1:4, o:o + n] = s_ea[a:b].T
        dlv2[c, o:o + n] = s_dl[a:b].astype(np.float32)

    gidx16 = np.stack([wrap_idx16(gidx[c]) for c in range(W)])
    ct2 = np.ascontiguousarray(
        np.repeat(ct2, 32, axis=1)).astype(BF16)         # [W, 128, EP2]

    dsf2 = np.zeros((W, 128, max(nb2, 1)), np.float32)
    for k in range(CH2):
        for (bcol, ws, items) in cb2[k]:
            dsf2[:, :, bcol] = dlv2[:, k * 128:(k + 1) * 128] - 128.0 * ws

    # x_augT packed (per core): window w at [32*(w%3):+4, (w//3)*128:+128]
    XCOLS = _ceil(NW, 3) // 3 * 128
    xaug = np.zeros((W, 128, XCOLS), np.float32)
    for c in range(W):
        xa = np.zeros((4, NLOCP), np.float32)
        xa[:DIM, :NLOC] = x[c * NLOC:(c + 1) * NLOC].T
        xa[3, :NLOC] = 1.0
        for ww in range(NW):
            xaug[c, 32 * (ww % 3):32 * (ww % 3) + 4,
                 (ww // 3) * 128:(ww // 3) * 128 + 128] = \
                xa[:, ww * 128:(ww + 1) * 128]

    # weights
    def stack_w(Wn, bn, in_c):
        S = np.zeros((128, HID), np.float32)
        B = bn.reshape(in_c, HID)
        S[0:in_c] = B
        for d in range(3):
            S[32 * (d + 1):32 * (d + 1) + in_c] = Wn[d].reshape(in_c, HID)
        return S.astype(BF16)

    w1stack = stack_w(np.asarray(params['Wn1'], np.float32),
                      np.asarray(params['bn1'], np.float32), DIM)
    w2stack = stack_w(np.asarray(params['Wn2'], np.float32),
                      np.asarray(params['bn2'], np.float32), HID)
    root1a = np.concatenate([np.asarray(params['root1'], np.float32),
                             np.asarray(params['b1'], np.float32)[None]], 0)
    root1a_p = np.zeros((68, HID), np.float32)
    for g in range(3):
        root1a_p[32 * g:32 * g + DIM] = root1a[:DIM]
        root1a_p[32 * g + 3] = root1a[DIM]
    root2a = np.concatenate([np.asarray(params['root2'], np.float32),
                             np.asarray(params['b2'], np.float32)[None]],
                            0).astype(BF16)
    wf1a = np.concatenate([np.asarray(params['Wf1'], np.float32),
                           np.asarray(params['bf1'], np.float32)[None]],
                          0).astype(BF16)
    wf2a = np.concatenate([np.asarray(params['Wf2'], np.float32),
                           np.asarray(params['bf2'], np.float32)[None]],
                          0).astype(BF16)
    iota = np.tile(np.arange(256, dtype=np.float32)[None, :],
                   (128, 1)).astype(BF16)
    ident = np.eye(128, dtype=np.float32)

    meta = dict(NLOC=NLOC, NLOCP=NLOCP, NP=NP, QS=QS, NW=NW,
                EP1=EP1, CH1=CH1, TB1=TB1, nb1=nb1, cb1=cb1, lp1=lp1, wp1=wp1,
                EP2=EP2, CH2=CH2, nb2=nb2, cb2=cb2, lp2=lp2, wp2=wp2,
                coff1=[int(v) for v in coff1], R1=[int(v) for v in R1],
                qoff2=[int(v) for v in qoff],
                XCOLS=XCOLS, calls=calls, W=W, HID=HID, DIM=DIM)

    shared = dict(w1stack=w1stack, w2stack=w2stack,
                  root1a=root1a_p, root2a=root2a, wf1a=wf1a, wf2a=wf2a,
                  iota=iota, ident=ident)
    in_maps = []
    for c in range(W):
        m = dict(shared)
        m['g1'] = g1[c]
        m['ct1'] = ct1[c]
        m['dsf1'] = dsf1[c]
        m['gidx'] = gidx16[c]
        m['ct2'] = ct2[c]
        m['dsf2'] = dsf2[c]
        m['xaug'] = xaug[c]
        in_maps.append(m)
    return in_maps, meta


def build_bass(meta):
    W, HID = meta['W'], meta['HID']
    NLOCP, NP, QS, NW = meta['NLOCP'], meta['NP'], meta['QS'], meta['NW']
    EP1, CH1, nb1 = meta['EP1'], meta['CH1'], meta['nb1']
    EP2, CH2, nb2 = meta['EP2'], meta['CH2'], meta['nb2']
    cb1, lp1, wp1 = meta['cb1'], meta['lp1'], meta['wp1']
    cb2, lp2, wp2 = meta['cb2'], meta['lp2'], meta['wp2']
    XCOLS, calls = meta['XCOLS'], meta['calls']
    coff1, R1, qoff2 = meta['coff1'], meta['R1'], meta['qoff2']
    RANKS_Q = QS // 128
    f32, bf16, i16 = mybir.dt.float32, mybir.dt.bfloat16, mybir.dt.int16

    nc = bacc.Bacc("TRN2", target_bir_lowering=False, debug=False,
                   num_devices=W, enable_asserts=False)

    # I/O ------------------------------------------------------------------
    g1_d = nc.dram_tensor("g1", [128, EP1], bf16, kind="ExternalInput")
    ct1_d = nc.dram_tensor("ct1", [128, EP1], bf16, kind="ExternalInput")
    dsf1_d = nc.dram_tensor("dsf1", [128, max(nb1, 1)], f32,
                            kind="ExternalInput")
    gidx_d = nc.dram_tensor("gidx", [128, EP2 // 16], i16,
                            kind="ExternalInput")
    ct2_d = nc.dram_tensor("ct2", [128, EP2], bf16, kind="ExternalInput")
    dsf2_d = nc.dram_tensor("dsf2", [128, max(nb2, 1)], f32,
                            kind="ExternalInput")
    xaug_d = nc.dram_tensor("xaug", [128, XCOLS], f32, kind="ExternalInput")
    w1_d = nc.dram_tensor("w1stack", [128, HID], bf16, kind="ExternalInput")
    w2_d = nc.dram_tensor("w2stack", [128, HID], bf16, kind="ExternalInput")
    r1_d = nc.dram_tensor("root1a", [68, HID], f32, kind="ExternalInput")
    r2_d = nc.dram_tensor("root2a", [33, HID], bf16, kind="ExternalInput")
    wf1_d = nc.dram_tensor("wf1a", [33, HID], bf16, kind="ExternalInput")
    wf2_d = nc.dram_tensor("wf2a", [33, 1], bf16, kind="ExternalInput")
    iota_d = nc.dram_tensor("iota", [128, 256], bf16, kind="ExternalInput")
    id_d = nc.dram_tensor("ident", [128, 128], f32, kind="ExternalInput")
    out_d = nc.dram_tensor("out", [1, NLOCP], f32, kind="ExternalOutput")

    cc_in = nc.dram_tensor("cc_in", [NLOCP, HID], bf16)
    cc_out = nc.dram_tensor("cc_out", [NP, HID], bf16, addr_space="Shared")

    ctx = ExitStack()
    with tile.TileContext(nc) as tc:
      with ctx:
        const = ctx.enter_context(tc.tile_pool(name="const", bufs=1))
        big = ctx.enter_context(tc.tile_pool(name="big", bufs=1))
        pipe = ctx.enter_context(tc.tile_pool(name="pipe", bufs=2))
        ohp = ctx.enter_context(tc.tile_pool(name="ohp", bufs=12))
        tabp = ctx.enter_context(tc.tile_pool(name="tabp", bufs=1))

        nc.gpsimd.load_library(library_config.mlp)

        # ---- constant loads ----
        def load(pool, dram, shape, dtype):
            t = pool.tile(shape, dtype, tag="c_" + dram.name)
            nc.sync.dma_start(out=t[:], in_=dram[:, :])
            return t

        dsf1_s = load(const, dsf1_d, [128, max(nb1, 1)], f32)
        gidx_s = load(const, gidx_d, [128, EP2 // 16], i16)
        dsf2_s = load(const, dsf2_d, [128, max(nb2, 1)], f32)
        xaug_s = load(const, xaug_d, [128, XCOLS], f32)
        w1_s = load(const, w1_d, [128, HID], bf16)
        w2_s = load(const, w2_d, [128, HID], bf16)
        r1_s = load(const, r1_d, [68, HID], f32)
        r2_s = load(const, r2_d, [33, HID], bf16)
        wf1_s = load(const, wf1_d, [33, HID], bf16)
        wf2_s = load(const, wf2_d, [33, 1], bf16)
        iota_s = load(const, iota_d, [128, 256], bf16)
        id_s = load(const, id_d, [128, 128], f32)

        CHMAX = max(CH1, CH2)
        msg_s = big.tile([128, CHMAX * 32], bf16)   # all msgs of one layer
        h1c_s = big.tile([128, NW * 32], bf16)      # compact local h1
        h1T_s = big.tile([33, NLOCP], bf16)         # h1^T augmented
        h2T_s = h1T_s                               # reused for h2^T (WAR-safe)
        nc.vector.memset(h1T_s[32:33, :], 1.0)

        def do_msg(ps1, t, Gt, Ct, b, wstack):
            """One 512-edge block of pass 1: messages into msg_s."""
            Ft = pipe.tile([128, 512], bf16, tag="F")
            nc.vector.tensor_tensor(Ft[:], Gt[:, 512 * b:512 * b + 512],
                                    Ct[:, 512 * b:512 * b + 512], ALU.mult)
            Mp = ps1.tile([128, 128], f32, tag="M")
            for j in range(4):
                nc.tensor.matmul(Mp[:, 32 * j:32 * j + 32],
                                 Ft[:, 128 * j:128 * j + 128],
                                 wstack[:], start=True, stop=True)
            nc.scalar.activation(msg_s[:, t * 128:(t + 1) * 128],
                                 Mp[:], AF.Copy)

        def do_agg(aggp, t, cbuilds, lastp, dsf_s):
            """Inline aggregation of block t's four chunks."""
            for kk in range(4):
                k = 4 * t + kk
                for (bcol, ws, items) in cbuilds[k]:
                    wd = 128 * len(items)
                    oh = ohp.tile([128, 256], bf16, tag="oh")
                    nc.vector.tensor_scalar(oh[:, 0:wd], iota_s[:, 0:wd],
                                            dsf_s[:, bcol:bcol + 1], 0.0,
                                            ALU.is_equal, ALU.bypass)
                    for (ww, sl) in items:
                        a = aggp[:, 32 * ww:32 * ww + 32]
                        nc.tensor.matmul(
                            a, oh[:, 128 * sl:128 * sl + 128],
                            msg_s[:, 32 * k:32 * k + 32], start=False,
                            stop=(lastp[ww] == (k, bcol, sl)),
                            skip_group_check=True)

        def roots(layer, aggp, winp):
            for ww in range(NW):
                if layer == 1:
                    g = 32 * (ww % 3)
                    lhs = xaug_s[g:g + 4,
                                 (ww // 3) * 128:(ww // 3) * 128 + 128]
                    rr = r1_s[g:g + 4, :]
                else:
                    lhs = h1T_s[:, ww * 128:(ww + 1) * 128]
                    rr = r2_s[:]
                a = aggp[:, 32 * ww:32 * ww + 32]
                nc.tensor.matmul(a, lhs, rr, start=False,
                                 stop=(winp[ww] == 0), skip_group_check=True)


        def make_bywin(cbuilds, CH, klo=0, khi=None):
            khi = CH if khi is None else khi
            bywin = [[] for _ in range(NW)]
            for k in range(klo, khi):
                for (bcol, ws, items) in cbuilds[k]:
                    for (ww, sl) in items:
                        bywin[ww].append((k, bcol, sl, 128 * len(items)))
            return bywin

        def emit_root(layer, aggp, ww, stop):
            if layer == 1:
                g = 32 * (ww % 3)
                lhs = xaug_s[g:g + 4,
                             (ww // 3) * 128:(ww // 3) * 128 + 128]
                rr = r1_s[g:g + 4, :]
            else:
                lhs = h1T_s[:, ww * 128:(ww + 1) * 128]
                rr = r2_s[:]
            a = aggp[:, 32 * ww:32 * ww + 32]
            nc.tensor.matmul(a, lhs, rr, start=False, stop=stop,
                             skip_group_check=True)

        def emit_win(aggp, ww, plist, dsf_s, lastp):
            a = aggp[:, 32 * ww:32 * ww + 32]
            for (k, bcol, sl, wd) in plist:
                oh = ohp.tile([128, 256], bf16, tag="oh")
                nc.vector.tensor_scalar(oh[:, 0:wd], iota_s[:, 0:wd],
                                        dsf_s[:, bcol:bcol + 1], 0.0,
                                        ALU.is_equal, ALU.bypass)
                nc.tensor.matmul(a, oh[:, 128 * sl:128 * sl + 128],
                                 msg_s[:, 32 * k:32 * k + 32],
                                 start=False,
                                 stop=(lastp[ww] == (k, bcol, sl)),
                                 skip_group_check=True)

        def tail_a(layer, aggp, ps):
            """relu + transpose into hT (and h1c on layer 1)."""
            hT = h1T_s if layer == 1 else h2T_s
            for w0 in range(0, NW, 4):
                nsub = min(4, NW - w0)
                trp = ps.tile([32, 512], f32, tag="tr")
                for i in range(nsub):
                    ww = w0 + i
                    a = aggp[:, 32 * ww:32 * ww + 32]
                    hf = pipe.tile([128, 32], f32, tag="hf")
                    nc.scalar.activation(hf[:], a, AF.Relu)
                    nc.tensor.transpose(trp[:, 128 * i:128 * i + 128],
                                        hf[:], id_s[:])
                span = 128 * nsub
                nc.scalar.activation(hT[0:32, 128 * w0:128 * w0 + span],
                                     trp[:, 0:span], AF.Copy)

        def tail_fc():
            with tc.tile_pool(name="psf", bufs=2, space="PSUM") as psf:
                for w0 in range(0, NW, 4):
                    span = 128 * min(4, NW - w0)
                    f1 = psf.tile([32, 512], f32, tag="f1")
                    nc.tensor.matmul(f1[:, 0:span], wf1_s[:],
                                     h2T_s[:, 128 * w0:128 * w0 + span],
                                     start=True, stop=True)
                    h3t = pipe.tile([33, 512], bf16, tag="h3t")
                    nc.scalar.activation(h3t[0:32, 0:span],
                                         f1[:, 0:span], AF.Relu)
                    nc.vector.memset(h3t[32:33, 0:span], 1.0)
                    f2 = psf.tile([1, 512], f32, tag="f2")
                    nc.tensor.matmul(f2[:, 0:span], wf2_s[:],
                                     h3t[:, 0:span], start=True, stop=True)
                    ot = pipe.tile([1, 512], f32, tag="ot")
                    nc.scalar.activation(ot[:, 0:span], f2[:, 0:span],
                                         AF.Copy)
                    nc.sync.dma_start(
                        out=out_d[:, 128 * w0:128 * w0 + span],
                        in_=ot[:, 0:span])

        # ================= layer 1 =================
        with nc.named_scope("l1"), \
             tc.tile_pool(name="agg1", bufs=1, space="PSUM") as pa1:
            aggp = pa1.tile([128, NW * 32], f32, tag="agg")
            nc.vector.memset(aggp[:], 0.0)
            bywin1 = make_bywin(cb1, CH1)
            sched1 = {}
            for ww in range(NW):
                lb = (coff1[ww] + R1[ww] - 1) // 512
                sched1.setdefault(lb, []).append(ww)
            with tc.tile_pool(name="ps1", bufs=1, space="PSUM") as ps1:
                for eoff in range(0, EP1, 2048):
                    csz = min(2048, EP1 - eoff)
                    Gt = pipe.tile([128, 2048], bf16, tag="G")
                    nc.sync.dma_start(out=Gt[:, 0:csz],
                                      in_=g1_d[:, eoff:eoff + csz])
                    Ct = pipe.tile([128, 2048], bf16, tag="C4")
                    nc.sync.dma_start(out=Ct[:, 0:csz],
                                      in_=ct1_d[:, eoff:eoff + csz])
                    for b in range(csz // 512):
                        t = (eoff + b * 512) // 512
                        do_msg(ps1, t, Gt, Ct, b, w1_s)
                        for ww in sched1.get(t, []):
                            emit_root(1, aggp, ww, wp1[ww] == 0)
                            emit_win(aggp, ww, bywin1[ww], dsf1_s, lp1)
                            nc.scalar.activation(
                                h1c_s[:, 32 * ww:32 * ww + 32],
                                aggp[:, 32 * ww:32 * ww + 32], AF.Relu)
            # ship compact h1 (deps: inline h1c copies only)
            nc.sync.dma_start(
                out=cc_in.ap().rearrange("(w p) h -> p w h", p=128),
                in_=h1c_s[:].rearrange("p (w h) -> p w h", h=HID))
            with tc.tile_pool(name="pst1", bufs=1, space="PSUM") as pst:
                tail_a(1, aggp, pst)

        with nc.named_scope("allgather"):
            nc.gpsimd.collective_compute(
                "AllGather", ALU.bypass, replica_groups=[list(range(W))],
                ins=[cc_in.ap().opt()], outs=[cc_out.ap().opt()])

        # ================= layer 2 =================
        with nc.named_scope("l2"), \
             tc.tile_pool(name="agg2", bufs=1, space="PSUM") as pa2:
            aggp = pa2.tile([128, NW * 32], f32, tag="agg")
            nc.vector.memset(aggp[:], 0.0)
            bywin2q = [make_bywin(cb2, CH2, qoff2[q] // 128,
                                  qoff2[q + 1] // 128) for q in range(4)]
            with tc.tile_pool(name="ps1b", bufs=1, space="PSUM") as ps1b:
                for qq in range(4):
                    cw = tabp.tile([128, RANKS_Q * 32], bf16, tag="cw")
                    nc.sync.dma_start(
                        out=cw[:].rearrange("p (r h) -> p r h", h=HID),
                        in_=cc_out.ap().rearrange("(q r p) h -> q p r h",
                                                  q=4, p=128)[qq])
                    tq = tabp.tile([128, RANKS_Q * 128], bf16, tag="tq")
                    tq4 = tq[:].rearrange("p (r d h) -> p r d h", d=4, h=HID)
                    cw3 = cw[:].rearrange("p (r h) -> p r h", h=HID)
                    for d in range(4):
                        nc.vector.tensor_copy(tq4[:, :, d, :], cw3)
                    for (cq, eoff, csz) in calls:
                        if cq != qq:
                            continue
                        Gt = pipe.tile([128, 2048], bf16, tag="G")
                        g3 = Gt[:, 0:csz].rearrange("p (o n) -> p o n", o=1)
                        nc.gpsimd.dma_gather(
                            g3, tq[:],
                            gidx_s[:, eoff // 16:(eoff + csz) // 16],
                            csz, csz, 128, transpose=True,
                            single_packet=False,
                            sbuf_tokens_per_rank=128,
                            sbuf_free_dim_per_rank=256,
                            sbuf_free_dim_pad_per_rank=0, sbuf_byte_offset=0)
                        Ct = pipe.tile([128, 2048], bf16, tag="C4")
                        nc.sync.dma_start(out=Ct[:, 0:csz],
                                          in_=ct2_d[:, eoff:eoff + csz])
                        for b in range(csz // 512):
                            t = (eoff + b * 512) // 512
                            do_msg(ps1b, t, Gt, Ct, b, w2_s)
                    if qq < 3:
                        for ww in range(NW):
                            if qq == 0:
                                emit_root(2, aggp, ww, wp2[ww] == 0)
                            emit_win(aggp, ww, bywin2q[qq][ww], dsf2_s, lp2)
            # final quarter's aggregation fused with the relu/transpose tail
            with tc.tile_pool(name="pst2", bufs=1, space="PSUM") as pst:
                for w0 in range(0, NW, 4):
                    nsub = min(4, NW - w0)
                    for i in range(nsub):
                        emit_win(aggp, w0 + i, bywin2q[3][w0 + i],
                                 dsf2_s, lp2)
                    trp = pst.tile([32, 512], f32, tag="tr")
                    for i in range(nsub):
                        ww = w0 + i
                        a = aggp[:, 32 * ww:32 * ww + 32]
                        hf = pipe.tile([128, 32], f32, tag="hf")
                        nc.scalar.activation(hf[:], a, AF.Relu)
                        nc.tensor.transpose(trp[:, 128 * i:128 * i + 128],
                                            hf[:], id_s[:])
                    span = 128 * nsub
                    nc.scalar.activation(
                        h2T_s[0:32, 128 * w0:128 * w0 + span],
                        trp[:, 0:span], AF.Copy)
        tail_fc()
    return nc


def run_kernel(inputs, cfg=None, trace=False):
    cfg = cfg or FULL_CFG
    W = cfg['W']
    params = {k: inputs[k] for k in
              ('Wn1', 'bn1', 'root1', 'b1', 'Wn2', 'bn2', 'root2', 'b2',
               'Wf1', 'bf1', 'Wf2', 'bf2')}
    in_maps, meta = host_prep(inputs['x'], inputs['edge_index'],
                              inputs['edge_attr'], params, cfg)
    nc = build_bass(meta)
    nc.finalize()
    res = run_bass_kernel_spmd(nc, in_maps, core_ids=list(range(W)),
                               trace=trace)
    NLOC = meta['NLOC']
    out = np.zeros((cfg['N'], 1), np.float32)
    for c in range(W):
        out[c * NLOC:(c + 1) * NLOC, 0] = res.results[c]['out'][0, :NLOC]
    return out, res


def kernel(**inputs):
    out, _ = run_kernel(inputs)
    return out


# revision 46
# speedup vs baseline: 1.2249x; 1.0025x over previous
"""NNConv (gnn_message_passing) SPMD kernel for 8 trn2 NeuronCores.

Strategy (dst-sharded, both layers):
  - Each core owns a contiguous range of NLOC nodes (dst sharding). Edges are
    assigned to the core owning their dst.
  - msg = kron([1, ea], h_src) @ Wstack  (the NNConv per-edge weight matmul
    factorizes into a plain matmul over a 128-wide feature built from
    c = [1, ea0, ea1, ea2] outer h_src).
  - Layer 1 source features: host pre-expands x[src] into a [128, EP1] bf16
    stream (pure indexing/layout), streamed sequentially from DRAM — no
    device-side gather. L1 edges are sorted by (dst window, dst), minimizing
    aggregation one-hot pairs.
  - Layer 2 source features: DMA transpose-gather (256B rows, bf16,
    replicated 4x) from an SBUF-resident table built from the allgathered
    compact h1. L2 edges sorted by (src-quarter, dst window, dst).
  - Aggregation (segment sum over dst): one-hot PE matmuls into PSUM-resident
    per-window accumulators (window = 128 dst nodes), fused with the root-term
    matmul and ReLU.
  - One AllGather (compact h1, bf16) between the layers.
  - Edge layout is made identical across cores via shared R-tables
    (cell counts padded to the max over cores), so a single SPMD program
    works for all 8 cores.
"""

import sys

if '/opt/trn_rl_repo' not in sys.path:
    sys.path.insert(0, '/opt/trn_rl_repo')

from contextlib import ExitStack

import ml_dtypes
import numpy as np

import concourse.bacc as bacc
import concourse.bass as bass
from concourse import mybir, tile
from concourse.bass_utils import run_bass_kernel_spmd
from concourse import library_config

BF16 = ml_dtypes.bfloat16
AF = mybir.ActivationFunctionType
ALU = mybir.AluOpType

FULL_CFG = dict(N=100000, E=400000, W=8, DIM=3, HID=32)


def _ceil(a, b):
    return -(-a // b) * b


def make_geom(N, W):
    NLOC = N // W
    NLOCP = _ceil(NLOC, 128)
    NP = W * NLOCP
    assert NP % 4 == 0
    QS = NP // 4          # table rows per src-quarter
    assert QS % 128 == 0
    NW = NLOCP // 128     # dst windows per core
    return NLOC, NLOCP, NP, QS, NW


def wrap_idx16(idx):
    """Edge i -> [i%16, i//16], tiled to 128 partitions (int16)."""
    a = np.asarray(idx, np.int16).reshape(-1, 16).T
    return np.tile(a, (8, 1))


def _layout(cnt_by_core_cell, n_cells, pad_to=512):
    """Shared cell layout: R[cell] = max count over cores, total padded."""
    R = cnt_by_core_cell.max(axis=0).astype(np.int64)   # [n_cells]
    tot = int(R.sum())
    R[n_cells - 1] += _ceil(tot, pad_to) - tot
    coff = np.concatenate([[0], np.cumsum(R)])
    EP = int(coff[-1])
    return R, coff[:-1], EP


def _pairs(cell_list, CH, NW):
    """Chunk-level one-hot build plan.

    Returns (nbuilds, chunk_builds, last_pair, win_pairs, touched) where
    chunk_builds[k] = [(bcol, ws, [(ww, sl), ...]), ...] groups the windows
    chunk k touches into runs of <=2 adjacent windows (one 256-wide one-hot
    build per run), and last_pair[ww] = (k, bcol, sl) identifies the final
    accumulation into window ww (for the matmul stop flag).
    """
    pairs = [[] for _ in range(CH)]
    for (ww, off, ln) in cell_list:
        if ln == 0:
            continue
        k0, k1 = off // 128, (off + ln - 1) // 128
        for k in range(k0, k1 + 1):
            pairs[k].append(ww)
    chunk_builds = []
    nbuilds = 0
    last_pair = {}
    win_pairs = [0] * NW
    for k in range(CH):
        ws_list = sorted(set(pairs[k]))
        builds = []
        i = 0
        while i < len(ws_list):
            if i + 1 < len(ws_list) and ws_list[i + 1] == ws_list[i] + 1:
                run = [ws_list[i], ws_list[i + 1]]
                i += 2
            else:
                run = [ws_list[i]]
                i += 1
            items = [(ww, sl) for sl, ww in enumerate(run)]
            builds.append((nbuilds, run[0], items))
            for (ww, sl) in items:
                last_pair[ww] = (k, nbuilds, sl)
                win_pairs[ww] += 1
            nbuilds += 1
        chunk_builds.append(builds)
    return nbuilds, chunk_builds, last_pair, win_pairs


def host_prep(x, edge_index, edge_attr, params, cfg):
    """Build per-core input arrays + shared structural metadata."""
    N, E, W, DIM, HID = cfg['N'], cfg['E'], cfg['W'], cfg['DIM'], cfg['HID']
    NLOC, NLOCP, NP, QS, NW = make_geom(N, W)

    src = np.asarray(edge_index[0], np.int64)
    dst = np.asarray(edge_index[1], np.int64)
    ea = np.asarray(edge_attr, np.float32)

    tr = (src // NLOC) * NLOCP + (src % NLOC)    # gather-table row
    core = dst // NLOC
    q = tr // QS
    dl = dst % NLOC                              # dst local id
    w = dl // 128                                # dst window

    x = np.asarray(x, np.float32)

    # x4 table rows: [x(3)|0]*4 per node (for host expansion + L2 analog)
    x4 = np.zeros((NP, 128), np.float32)
    rows = np.arange(NP)
    rc, rl = rows // NLOCP, rows % NLOCP
    valid = rl < NLOC
    nid = np.clip(rc * NLOC + rl, 0, N - 1)
    for d in range(4):
        x4[valid, 32 * d:32 * d + DIM] = x[nid[valid]]
    x4 = x4.astype(BF16)

    # ===== layer 1 layout: cells = dst windows ============================
    key1 = core * NW + w
    cnt1 = np.bincount(key1, minlength=W * NW).reshape(W, NW)
    R1, coff1, EP1 = _layout(cnt1, NW)
    CH1, TB1 = EP1 // 128, EP1 // 512
    cell_list1 = [(ww, int(coff1[ww]), int(R1[ww])) for ww in range(NW)]
    nb1, cb1, lp1, wp1 = _pairs(cell_list1, CH1, NW)

    order1 = np.lexsort((dl, w, core))
    o_core = core[order1]
    o_w = w[order1]
    o_tr = tr[order1]
    o_dl = dl[order1]
    o_ea = ea[order1]

    g1 = np.zeros((W, EP1, 128), BF16)           # host-expanded x4[src]
    ct1 = np.zeros((W, 4, EP1), np.float32)
    dlv1 = np.full((W, EP1), -10000.0, np.float32)

    ckey = o_core * NW + o_w
    gs = np.flatnonzero(np.r_[True, ckey[1:] != ckey[:-1]])
    ge = np.r_[gs[1:], len(ckey)]
    for a, b in zip(gs, ge):
        c = int(o_core[a]); ww = int(o_w[a])
        o = int(coff1[ww]); n = b - a
        g1[c, o:o + n] = x4[o_tr[a:b]]
        ct1[c, 0, o:o + n] = 1.0
        ct1[c, 1:4, o:o + n] = o_ea[a:b].T
        dlv1[c, o:o + n] = o_dl[a:b].astype(np.float32)

    g1 = np.ascontiguousarray(g1.transpose(0, 2, 1))     # [W, 128, EP1]
    # expand c rows 32x into the 128-wide kron layout (pure replication)
    ct1 = np.ascontiguousarray(
        np.repeat(ct1, 32, axis=1)).astype(BF16)         # [W, 128, EP1]

    dsf1 = np.zeros((W, 128, max(nb1, 1)), np.float32)
    for k in range(CH1):
        for (bcol, ws, items) in cb1[k]:
            dsf1[:, :, bcol] = dlv1[:, k * 128:(k + 1) * 128] - 128.0 * ws

    # ===== layer 2 layout: cells = (src quarter, dst window) ==============
    key2 = (core * 4 + q) * NW + w
    cnt2 = np.bincount(key2, minlength=W * 4 * NW).reshape(W, 4 * NW)
    # pad each quarter's edge total to a 512 multiple (extend last cell)
    R2 = cnt2.max(axis=0).astype(np.int64).reshape(4, NW)
    for qq in range(4):
        tot = int(R2[qq].sum())
        R2[qq, NW - 1] += _ceil(tot, 512) - tot
    qsz = R2.sum(axis=1)
    qoff = np.concatenate([[0], np.cumsum(qsz)])
    EP2 = int(qsz.sum())
    CH2 = EP2 // 128
    coff2 = np.zeros((4, NW), np.int64)
    run = 0
    cell_list2 = []
    for qq in range(4):
        for ww in range(NW):
            coff2[qq, ww] = run
            cell_list2.append((ww, run, int(R2[qq, ww])))
            run += int(R2[qq, ww])
    assert run == EP2
    nb2, cb2, lp2, wp2 = _pairs(cell_list2, CH2, NW)

    calls = []                                   # (q, edge_off, size)
    for qq in range(4):
        o = 0
        while o < qsz[qq]:
            s = min(2048, int(qsz[qq]) - o)
            calls.append((qq, int(qoff[qq]) + o, s))
            o += s

    order2 = np.lexsort((dl, w, q, core))
    s_core = core[order2]
    s_q = q[order2]
    s_w = w[order2]
    s_tr = tr[order2]
    s_dl = dl[order2]
    s_ea = ea[order2]

    gidx = np.zeros((W, EP2), np.int64)
    ct2 = np.zeros((W, 4, EP2), np.float32)
    dlv2 = np.full((W, EP2), -10000.0, np.float32)

    ckey2 = (s_core * 4 + s_q) * NW + s_w
    gs = np.flatnonzero(np.r_[True, ckey2[1:] != ckey2[:-1]])
    ge = np.r_[gs[1:], len(ckey2)]
    for a, b in zip(gs, ge):
        c = int(s_core[a]); qq = int(s_q[a]); ww = int(s_w[a])
        o = int(coff2[qq, ww]); n = b - a
        gidx[c, o:o + n] = s_tr[a:b] - qq * QS
        ct2[c, 0, o:o + n] = 1.0
        ct2[c, 1:4, o:o + n] = s_ea[a:b].T
        dlv2[c, o:o + n] = s_dl[a:b].astype(np.float32)

    gidx16 = np.stack([wrap_idx16(gidx[c]) for c in range(W)])
    ct2 = np.ascontiguousarray(
        np.repeat(ct2, 32, axis=1)).astype(BF16)         # [W, 128, EP2]

    dsf2 = np.zeros((W, 128, max(nb2, 1)), np.float32)
    for k in range(CH2):
        for (bcol, ws, items) in cb2[k]:
            dsf2[:, :, bcol] = dlv2[:, k * 128:(k + 1) * 128] - 128.0 * ws

    # x_augT packed (per core): window w at [32*(w%3):+4, (w//3)*128:+128]
    XCOLS = _ceil(NW, 3) // 3 * 128
    xaug = np.zeros((W, 128, XCOLS), np.float32)
    for c in range(W):
        xa = np.zeros((4, NLOCP), np.float32)
        xa[:DIM, :NLOC] = x[c * NLOC:(c + 1) * NLOC].T
        xa[3, :NLOC] = 1.0
        for ww in range(NW):
            xaug[c, 32 * (ww % 3):32 * (ww % 3) + 4,
                 (ww // 3) * 128:(ww // 3) * 128 + 128] = \
                xa[:, ww * 128:(ww + 1) * 128]

    # weights
    def stack_w(Wn, bn, in_c):
        S = np.zeros((128, HID), np.float32)
        B = bn.reshape(in_c, HID)
        S[0:in_c] = B
        for d in range(3):
            S[32 * (d + 1):32 * (d + 1) + in_c] = Wn[d].reshape(in_c, HID)
        return S.astype(BF16)

    w1stack = stack_w(np.asarray(params['Wn1'], np.float32),
                      np.asarray(params['bn1'], np.float32), DIM)
    w2stack = stack_w(np.asarray(params['Wn2'], np.float32),
                      np.asarray(params['bn2'], np.float32), HID)
    root1a = np.concatenate([np.asarray(params['root1'], np.float32),
                             np.asarray(params['b1'], np.float32)[None]], 0)
    root1a_p = np.zeros((68, HID), np.float32)
    for g in range(3):
        root1a_p[32 * g:32 * g + DIM] = root1a[:DIM]
        root1a_p[32 * g + 3] = root1a[DIM]
    root2a = np.concatenate([np.asarray(params['root2'], np.float32),
                             np.asarray(params['b2'], np.float32)[None]],
                            0).astype(BF16)
    wf1a = np.concatenate([np.asarray(params['Wf1'], np.float32),
                           np.asarray(params['bf1'], np.float32)[None]],
                          0).astype(BF16)
    wf2a = np.concatenate([np.asarray(params['Wf2'], np.float32),
                           np.asarray(params['bf2'], np.float32)[None]],
                          0).astype(BF16)
    iota = np.tile(np.arange(256, dtype=np.float32)[None, :],
                   (128, 1)).astype(BF16)
    ident = np.eye(128, dtype=np.float32)

    meta = dict(NLOC=NLOC, NLOCP=NLOCP, NP=NP, QS=QS, NW=NW,
                EP1=EP1, CH1=CH1, TB1=TB1, nb1=nb1, cb1=cb1, lp1=lp1, wp1=wp1,
                EP2=EP2, CH2=CH2, nb2=nb2, cb2=cb2, lp2=lp2, wp2=wp2,
                coff1=[int(v) for v in coff1], R1=[int(v) for v in R1],
                qoff2=[int(v) for v in qoff],
                XCOLS=XCOLS, calls=calls, W=W, HID=HID, DIM=DIM)

    shared = dict(w1stack=w1stack, w2stack=w2stack,
                  root1a=root1a_p, root2a=root2a, wf1a=wf1a, wf2a=wf2a,
                  iota=iota, ident=ident)
    in_maps = []
    for c in range(W):
        m = dict(shared)
        m['g1'] = g1[c]
        m['ct1'] = ct1[c]
        m['dsf1'] = dsf1[c]
        m['gidx'] = gidx16[c]
        m['ct2'] = ct2[c]
        m['dsf2'] = dsf2[c]
        m['xaug'] = xaug[c]
        in_maps.append(m)
    return in_maps, meta


def build_bass(meta):
    W, HID = meta['W'], meta['HID']
    NLOCP, NP, QS, NW = meta['NLOCP'], meta['NP'], meta['QS'], meta['NW']
    EP1, CH1, nb1 = meta['EP1'], meta['CH1'], meta['nb1']
    EP2, CH2, nb2 = meta['EP2'], meta['CH2'], meta['nb2']
    cb1, lp1, wp1 = meta['cb1'], meta['lp1'], meta['wp1']
    cb2, lp2, wp2 = meta['cb2'], meta['lp2'], meta['wp2']
    XCOLS, calls = meta['XCOLS'], meta['calls']
    coff1, R1, qoff2 = meta['coff1'], meta['R1'], meta['qoff2']
    RANKS_Q = QS // 128
    f32, bf16, i16 = mybir.dt.float32, mybir.dt.bfloat16, mybir.dt.int16

    nc = bacc.Bacc("TRN2", target_bir_lowering=False, debug=False,
                   num_devices=W, enable_asserts=False)

    # I/O ------------------------------------------------------------------
    g1_d = nc.dram_tensor("g1", [128, EP1], bf16, kind="ExternalInput")
    ct1_d = nc.dram_tensor("ct1", [128, EP1], bf16, kind="ExternalInput")
    dsf1_d = nc.dram_tensor("dsf1", [128, max(nb1, 1)], f32,
                            kind="ExternalInput")
    gidx_d = nc.dram_tensor("gidx", [128, EP2 // 16], i16,
                            kind="ExternalInput")
    ct2_d = nc.dram_tensor("ct2", [128, EP2], bf16, kind="ExternalInput")
    dsf2_d = nc.dram_tensor("dsf2", [128, max(nb2, 1)], f32,
                            kind="ExternalInput")
    xaug_d = nc.dram_tensor("xaug", [128, XCOLS], f32, kind="ExternalInput")
    w1_d = nc.dram_tensor("w1stack", [128, HID], bf16, kind="ExternalInput")
    w2_d = nc.dram_tensor("w2stack", [128, HID], bf16, kind="ExternalInput")
    r1_d = nc.dram_tensor("root1a", [68, HID], f32, kind="ExternalInput")
    r2_d = nc.dram_tensor("root2a", [33, HID], bf16, kind="ExternalInput")
    wf1_d = nc.dram_tensor("wf1a", [33, HID], bf16, kind="ExternalInput")
    wf2_d = nc.dram_tensor("wf2a", [33, 1], bf16, kind="ExternalInput")
    iota_d = nc.dram_tensor("iota", [128, 256], bf16, kind="ExternalInput")
    id_d = nc.dram_tensor("ident", [128, 128], f32, kind="ExternalInput")
    out_d = nc.dram_tensor("out", [1, NLOCP], f32, kind="ExternalOutput")

    cc_in = nc.dram_tensor("cc_in", [NLOCP, HID], bf16)
    cc_out = nc.dram_tensor("cc_out", [NP, HID], bf16, addr_space="Shared")

    ctx = ExitStack()
    with tile.TileContext(nc) as tc:
      with ctx:
        const = ctx.enter_context(tc.tile_pool(name="const", bufs=1))
        big = ctx.enter_context(tc.tile_pool(name="big", bufs=1))
        pipe = ctx.enter_context(tc.tile_pool(name="pipe", bufs=2))
        ohp = ctx.enter_context(tc.tile_pool(name="ohp", bufs=12))
        tabp = ctx.enter_context(tc.tile_pool(name="tabp", bufs=1))

        nc.gpsimd.load_library(library_config.mlp)

        # ---- constant loads ----
        def load(pool, dram, shape, dtype):
            t = pool.tile(shape, dtype, tag="c_" + dram.name)
            nc.sync.dma_start(out=t[:], in_=dram[:, :])
            return t

        dsf1_s = load(const, dsf1_d, [128, max(nb1, 1)], f32)
        gidx_s = load(const, gidx_d, [128, EP2 // 16], i16)
        dsf2_s = load(const, dsf2_d, [128, max(nb2, 1)], f32)
        xaug_s = load(const, xaug_d, [128, XCOLS], f32)
        w1_s = load(const, w1_d, [128, HID], bf16)
        w2_s = load(const, w2_d, [128, HID], bf16)
        r1_s = load(const, r1_d, [68, HID], f32)
        r2_s = load(const, r2_d, [33, HID], bf16)
        wf1_s = load(const, wf1_d, [33, HID], bf16)
        wf2_s = load(const, wf2_d, [33, 1], bf16)
        iota_s = load(const, iota_d, [128, 256], bf16)
        id_s = load(const, id_d, [128, 128], f32)

        CHMAX = max(CH1, CH2)
        msg_s = big.tile([128, CHMAX * 32], bf16)   # all msgs of one layer
        h1c_s = big.tile([128, NW * 32], bf16)      # compact local h1
        h1T_s = big.tile([33, NLOCP], bf16)         # h1^T augmented
        h2T_s = h1T_s                               # reused for h2^T (WAR-safe)
        nc.vector.memset(h1T_s[32:33, :], 1.0)

        def do_msg(ps1, t, Gt, Ct, b, wstack):
            """One 512-edge block of pass 1: messages into msg_s."""
            Ft = pipe.tile([128, 512], bf16, tag="F")
            nc.vector.tensor_tensor(Ft[:], Gt[:, 512 * b:512 * b + 512],
                                    Ct[:, 512 * b:512 * b + 512], ALU.mult)
            Mp = ps1.tile([128, 128], f32, tag="M")
            for j in range(4):
                nc.tensor.matmul(Mp[:, 32 * j:32 * j + 32],
                                 Ft[:, 128 * j:128 * j + 128],
                                 wstack[:], start=True, stop=True)
            nc.scalar.activation(msg_s[:, t * 128:(t + 1) * 128],
                                 Mp[:], AF.Copy)

        def do_agg(aggp, t, cbuilds, lastp, dsf_s):
            """Inline aggregation of block t's four chunks."""
            for kk in range(4):
                k = 4 * t + kk
                for (bcol, ws, items) in cbuilds[k]:
                    wd = 128 * len(items)
                    oh = ohp.tile([128, 256], bf16, tag="oh")
                    nc.vector.tensor_scalar(oh[:, 0:wd], iota_s[:, 0:wd],
                                            dsf_s[:, bcol:bcol + 1], 0.0,
                                            ALU.is_equal, ALU.bypass)
                    for (ww, sl) in items:
                        a = aggp[:, 32 * ww:32 * ww + 32]
                        nc.tensor.matmul(
                            a, oh[:, 128 * sl:128 * sl + 128],
                            msg_s[:, 32 * k:32 * k + 32], start=False,
                            stop=(lastp[ww] == (k, bcol, sl)),
                            skip_group_check=True)

        def roots(layer, aggp, winp):
            for ww in range(NW):
                if layer == 1:
                    g = 32 * (ww % 3)
                    lhs = xaug_s[g:g + 4,
                                 (ww // 3) * 128:(ww // 3) * 128 + 128]
                    rr = r1_s[g:g + 4, :]
                else:
                    lhs = h1T_s[:, ww * 128:(ww + 1) * 128]
                    rr = r2_s[:]
                a = aggp[:, 32 * ww:32 * ww + 32]
                nc.tensor.matmul(a, lhs, rr, start=False,
                                 stop=(winp[ww] == 0), skip_group_check=True)


        def make_bywin(cbuilds, CH, klo=0, khi=None):
            khi = CH if khi is None else khi
            bywin = [[] for _ in range(NW)]
            for k in range(klo, khi):
                for (bcol, ws, items) in cbuilds[k]:
                    for (ww, sl) in items:
                        bywin[ww].append((k, bcol, sl, 128 * len(items)))
            return bywin

        def emit_root(layer, aggp, ww, stop):
            if layer == 1:
                g = 32 * (ww % 3)
                lhs = xaug_s[g:g + 4,
                             (ww // 3) * 128:(ww // 3) * 128 + 128]
                rr = r1_s[g:g + 4, :]
            else:
                lhs = h1T_s[:, ww * 128:(ww + 1) * 128]
                rr = r2_s[:]
            a = aggp[:, 32 * ww:32 * ww + 32]
            nc.tensor.matmul(a, lhs, rr, start=False, stop=stop,
                             skip_group_check=True)

        def emit_win(aggp, ww, plist, dsf_s, lastp):
            a = aggp[:, 32 * ww:32 * ww + 32]
            for (k, bcol, sl, wd) in plist:
                oh = ohp.tile([128, 256], bf16, tag="oh")
                nc.vector.tensor_scalar(oh[:, 0:wd], iota_s[:, 0:wd],
                                        dsf_s[:, bcol:bcol + 1], 0.0,
                                        ALU.is_equal, ALU.bypass)
                nc.tensor.matmul(a, oh[:, 128 * sl:128 * sl + 128],
                                 msg_s[:, 32 * k:32 * k + 32],
                                 start=False,
                                 stop=(lastp[ww] == (k, bcol, sl)),
                                 skip_group_check=True)

        def tail_a(layer, aggp, ps):
            """relu + transpose into hT (and h1c on layer 1)."""
            hT = h1T_s if layer == 1 else h2T_s
            for w0 in range(0, NW, 4):
                nsub = min(4, NW - w0)
                trp = ps.tile([32, 512], f32, tag="tr")
                for i in range(nsub):
                    ww = w0 + i
                    a = aggp[:, 32 * ww:32 * ww + 32]
                    hf = pipe.tile([128, 32], f32, tag="hf")
                    nc.scalar.activation(hf[:], a, AF.Relu)
                    nc.tensor.transpose(trp[:, 128 * i:128 * i + 128],
                                        hf[:], id_s[:])
                span = 128 * nsub
                nc.scalar.activation(hT[0:32, 128 * w0:128 * w0 + span],
                                     trp[:, 0:span], AF.Copy)

        def tail_fc():
            with tc.tile_pool(name="psf", bufs=2, space="PSUM") as psf:
                for w0 in range(0, NW, 4):
                    span = 128 * min(4, NW - w0)
                    f1 = psf.tile([32, 512], f32, tag="f1")
                    nc.tensor.matmul(f1[:, 0:span], wf1_s[:],
                                     h2T_s[:, 128 * w0:128 * w0 + span],
                                     start=True, stop=True)
                    h3t = pipe.tile([33, 512], bf16, tag="h3t")
                    nc.scalar.activation(h3t[0:32, 0:span],
                                         f1[:, 0:span], AF.Relu)
                    nc.vector.memset(h3t[32:33, 0:span], 1.0)
                    f2 = psf.tile([1, 512], f32, tag="f2")
                    nc.tensor.matmul(f2[:, 0:span], wf2_s[:],
                                     h3t[:, 0:span], start=True, stop=True)
                    ot = pipe.tile([1, 512], f32, tag="ot")
                    nc.scalar.activation(ot[:, 0:span], f2[:, 0:span],
                                         AF.Copy)
                    nc.sync.dma_start(
                        out=out_d[:, 128 * w0:128 * w0 + span],
                        in_=ot[:, 0:span])

        # ================= layer 1 =================
        with nc.named_scope("l1"), \
             tc.tile_pool(name="agg1", bufs=1, space="PSUM") as pa1:
            aggp = pa1.tile([128, NW * 32], f32, tag="agg")
            nc.vector.memset(aggp[:], 0.0)
            bywin1 = make_bywin(cb1, CH1)
            sched1 = {}
            for ww in range(NW):
                lb = (coff1[ww] + R1[ww] - 1) // 512
                sched1.setdefault(lb, []).append(ww)
            with tc.tile_pool(name="ps1", bufs=1, space="PSUM") as ps1:
                for eoff in range(0, EP1, 2048):
                    csz = min(2048, EP1 - eoff)
                    Gt = pipe.tile([128, 2048], bf16, tag="G")
                    nc.sync.dma_start(out=Gt[:, 0:csz],
                                      in_=g1_d[:, eoff:eoff + csz])
                    Ct = pipe.tile([128, 2048], bf16, tag="C4")
                    nc.sync.dma_start(out=Ct[:, 0:csz],
                                      in_=ct1_d[:, eoff:eoff + csz])
                    for b in range(csz // 512):
                        t = (eoff + b * 512) // 512
                        do_msg(ps1, t, Gt, Ct, b, w1_s)
                        for ww in sched1.get(t, []):
                            emit_root(1, aggp, ww, wp1[ww] == 0)
                            emit_win(aggp, ww, bywin1[ww], dsf1_s, lp1)
                            nc.scalar.activation(
                                h1c_s[:, 32 * ww:32 * ww + 32],
                                aggp[:, 32 * ww:32 * ww + 32], AF.Relu)
            # ship compact h1 (deps: inline h1c copies only)
            nc.sync.dma_start(
                out=cc_in.ap().rearrange("(w p) h -> p w h", p=128),
                in_=h1c_s[:].rearrange("p (w h) -> p w h", h=HID))
            with tc.tile_pool(name="pst1", bufs=1, space="PSUM") as pst:
                tail_a(1, aggp, pst)

        with nc.named_scope("allgather"):
            nc.gpsimd.collective_compute(
                "AllGather", ALU.bypass, replica_groups=[list(range(W))],
                ins=[cc_in.ap().opt()], outs=[cc_out.ap().opt()])

        # ================= layer 2 =================
        with nc.named_scope("l2"), \
             tc.tile_pool(name="agg2", bufs=1, space="PSUM") as pa2:
            aggp = pa2.tile([128, NW * 32], f32, tag="agg")
            nc.vector.memset(aggp[:], 0.0)
            bywin2q = [make_bywin(cb2, CH2, qoff2[q] // 128,
                                  qoff2[q + 1] // 128) for q in range(4)]
            with tc.tile_pool(name="ps1b", bufs=1, space="PSUM") as ps1b:
                for qq in range(4):
                    cw = tabp.tile([128, RANKS_Q * 32], bf16, tag="cw")
                    nc.sync.dma_start(
                        out=cw[:].rearrange("p (r h) -> p r h", h=HID),
                        in_=cc_out.ap().rearrange("(q r p) h -> q p r h",
                                                  q=4, p=128)[qq])
                    tq = tabp.tile([128, RANKS_Q * 128], bf16, tag="tq")
                    tq4 = tq[:].rearrange("p (r d h) -> p r d h", d=4, h=HID)
                    cw3 = cw[:].rearrange("p (r h) -> p r h", h=HID)
                    for d in range(4):
                        nc.vector.tensor_copy(tq4[:, :, d, :], cw3)
                    for (cq, eoff, csz) in calls:
                        if cq != qq:
                            continue
                        Gt = pipe.tile([128, 2048], bf16, tag="G")
                        g3 = Gt[:, 0:csz].rearrange("p (o n) -> p o n", o=1)
                        nc.gpsimd.dma_gather(
                            g3, tq[:],
                            gidx_s[:, eoff // 16:(eoff + csz) // 16],
                            csz, csz, 128, transpose=True,
                            single_packet=False,
                            sbuf_tokens_per_rank=128,
                            sbuf_free_dim_per_rank=256,
                            sbuf_free_dim_pad_per_rank=0, sbuf_byte_offset=0)
                        Ct = pipe.tile([128, 2048], bf16, tag="C4")
                        nc.sync.dma_start(out=Ct[:, 0:csz],
                                          in_=ct2_d[:, eoff:eoff + csz])
                        for b in range(csz // 512):
                            t = (eoff + b * 512) // 512
                            do_msg(ps1b, t, Gt, Ct, b, w2_s)
                    if qq < 3:
                        for ww in range(NW):
                            if qq == 0:
                                emit_root(2, aggp, ww, wp2[ww] == 0)
                            emit_win(aggp, ww, bywin2q[qq][ww], dsf2_s, lp2)
            # final quarter's aggregation fused with the relu/transpose tail
            with tc.tile_pool(name="pst2", bufs=1, space="PSUM") as pst:
                for w0 in range(0, NW, 4):
                    nsub = min(4, NW - w0)
                    for i in range(nsub):
                        emit_win(aggp, w0 + i, bywin2q[3][w0 + i],
                                 dsf2_s, lp2)
                    trp = pst.tile([32, 512], f32, tag="tr")
                    for i in range(nsub):
                        ww = w0 + i
                        a = aggp[:, 32 * ww:32 * ww + 32]
                        hf = pipe.tile([128, 32], f32, tag="hf")
                        nc.scalar.activation(hf[:], a, AF.Relu)
                        nc.tensor.transpose(trp[:, 128 * i:128 * i + 128],
                                            hf[:], id_s[:])
                    span = 128 * nsub
                    nc.scalar.activation(
                        h2T_s[0:32, 128 * w0:128 * w0 + span],
                        trp[:, 0:span], AF.Copy)
        tail_fc()
    return nc


def run_kernel(inputs, cfg=None, trace=False):
    cfg = cfg or FULL_CFG
    W = cfg['W']
    params = {k: inputs[k] for k in
              ('Wn1', 'bn1', 'root1', 'b1', 'Wn2', 'bn2', 'root2', 'b2',
               'Wf1', 'bf1', 'Wf2', 'bf2')}
    in_maps, meta = host_prep(inputs['x'], inputs['edge_index'],
                              inputs['edge_attr'], params, cfg)
    nc = build_bass(meta)
    nc.finalize()
    res = run_bass_kernel_spmd(nc, in_maps, core_ids=list(range(W)),
                               trace=trace)
    NLOC = meta['NLOC']
    out = np.zeros((cfg['N'], 1), np.float32)
    for c in range(W):
        out[c * NLOC:(c + 1) * NLOC, 0] = res.results[c]['out'][0, :NLOC]
    return out, res


def kernel(**inputs):
    out, _ = run_kernel(inputs)
    return out


# revision 48
# speedup vs baseline: 1.2477x; 1.0186x over previous
"""NNConv (gnn_message_passing) SPMD kernel for 8 trn2 NeuronCores.

Strategy (dst-sharded, both layers):
  - Each core owns a contiguous range of NLOC nodes (dst sharding). Edges are
    assigned to the core owning their dst.
  - msg = kron([1, ea], h_src) @ Wstack  (the NNConv per-edge weight matmul
    factorizes into a plain matmul over a 128-wide feature built from
    c = [1, ea0, ea1, ea2] outer h_src).
  - Layer 1 source features: host pre-expands x[src] into a [128, EP1] bf16
    stream (pure indexing/layout), streamed sequentially from DRAM — no
    device-side gather. L1 edges are sorted by (dst window, dst), minimizing
    aggregation one-hot pairs.
  - Layer 2 source features: DMA transpose-gather (256B rows, bf16,
    replicated 4x) from an SBUF-resident table built from the allgathered
    compact h1. L2 edges sorted by (src-quarter, dst window, dst).
  - Aggregation (segment sum over dst): one-hot PE matmuls into PSUM-resident
    per-window accumulators (window = 128 dst nodes), fused with the root-term
    matmul and ReLU.
  - One AllGather (compact h1, bf16) between the layers.
  - Edge layout is made identical across cores via shared R-tables
    (cell counts padded to the max over cores), so a single SPMD program
    works for all 8 cores.
"""

import sys

if '/opt/trn_rl_repo' not in sys.path:
    sys.path.insert(0, '/opt/trn_rl_repo')

from contextlib import ExitStack

import ml_dtypes
import numpy as np

import concourse.bacc as bacc
import concourse.bass as bass
from concourse import mybir, tile
from concourse.bass_utils import run_bass_kernel_spmd
from concourse import library_config

BF16 = ml_dtypes.bfloat16
AF = mybir.ActivationFunctionType
ALU = mybir.AluOpType

FULL_CFG = dict(N=100000, E=400000, W=8, DIM=3, HID=32)


def _ceil(a, b):
    return -(-a // b) * b


def make_geom(N, W):
    NLOC = N // W
    NLOCP = _ceil(NLOC, 128)
    NP = W * NLOCP
    assert NP % 4 == 0
    QS = NP // 4          # table rows per src-quarter
    assert QS % 128 == 0
    NW = NLOCP // 128     # dst windows per core
    return NLOC, NLOCP, NP, QS, NW


def wrap_idx16(idx):
    """Edge i -> [i%16, i//16], tiled to 128 partitions (int16)."""
    a = np.asarray(idx, np.int16).reshape(-1, 16).T
    return np.tile(a, (8, 1))


def _layout(cnt_by_core_cell, n_cells, pad_to=512):
    """Shared cell layout: R[cell] = max count over cores, total padded."""
    R = cnt_by_core_cell.max(axis=0).astype(np.int64)   # [n_cells]
    tot = int(R.sum())
    R[n_cells - 1] += _ceil(tot, pad_to) - tot
    coff = np.concatenate([[0], np.cumsum(R)])
    EP = int(coff[-1])
    return R, coff[:-1], EP


def _pairs(cell_list, CH, NW):
    """Chunk-level one-hot build plan.

    Returns (nbuilds, chunk_builds, last_pair, win_pairs, touched) where
    chunk_builds[k] = [(bcol, ws, [(ww, sl), ...]), ...] groups the windows
    chunk k touches into runs of <=2 adjacent windows (one 256-wide one-hot
    build per run), and last_pair[ww] = (k, bcol, sl) identifies the final
    accumulation into window ww (for the matmul stop flag).
    """
    pairs = [[] for _ in range(CH)]
    for (ww, off, ln) in cell_list:
        if ln == 0:
            continue
        k0, k1 = off // 128, (off + ln - 1) // 128
        for k in range(k0, k1 + 1):
            pairs[k].append(ww)
    chunk_builds = []
    nbuilds = 0
    last_pair = {}
    win_pairs = [0] * NW
    for k in range(CH):
        ws_list = sorted(set(pairs[k]))
        builds = []
        i = 0
        while i < len(ws_list):
            if i + 1 < len(ws_list) and ws_list[i + 1] == ws_list[i] + 1:
                run = [ws_list[i], ws_list[i + 1]]
                i += 2
            else:
                run = [ws_list[i]]
                i += 1
            items = [(ww, sl) for sl, ww in enumerate(run)]
            builds.append((nbuilds, run[0], items))
            for (ww, sl) in items:
                last_pair[ww] = (k, nbuilds, sl)
                win_pairs[ww] += 1
            nbuilds += 1
        chunk_builds.append(builds)
    return nbuilds, chunk_builds, last_pair, win_pairs


def host_prep(x, edge_index, edge_attr, params, cfg):
    """Build per-core input arrays + shared structural metadata."""
    N, E, W, DIM, HID = cfg['N'], cfg['E'], cfg['W'], cfg['DIM'], cfg['HID']
    NLOC, NLOCP, NP, QS, NW = make_geom(N, W)

    src = np.asarray(edge_index[0], np.int64)
    dst = np.asarray(edge_index[1], np.int64)
    ea = np.asarray(edge_attr, np.float32)

    tr = (src // NLOC) * NLOCP + (src % NLOC)    # gather-table row
    core = dst // NLOC
    q = tr // QS
    dl = dst % NLOC                              # dst local id
    w = dl // 128                                # dst window

    x = np.asarray(x, np.float32)

    # x4 table rows: [x(3)|0]*4 per node (for host expansion + L2 analog)
    x4 = np.zeros((NP, 128), np.float32)
    rows = np.arange(NP)
    rc, rl = rows // NLOCP, rows % NLOCP
    valid = rl < NLOC
    nid = np.clip(rc * NLOC + rl, 0, N - 1)
    for d in range(4):
        x4[valid, 32 * d:32 * d + DIM] = x[nid[valid]]
    x4 = x4.astype(BF16)

    # ===== layer 1 layout: cells = dst windows ============================
    key1 = core * NW + w
    cnt1 = np.bincount(key1, minlength=W * NW).reshape(W, NW)
    R1, coff1, EP1 = _layout(cnt1, NW)
    CH1, TB1 = EP1 // 128, EP1 // 512
    cell_list1 = [(ww, int(coff1[ww]), int(R1[ww])) for ww in range(NW)]
    nb1, cb1, lp1, wp1 = _pairs(cell_list1, CH1, NW)

    order1 = np.lexsort((dl, w, core))
    o_core = core[order1]
    o_w = w[order1]
    o_tr = tr[order1]
    o_dl = dl[order1]
    o_ea = ea[order1]

    g1 = np.zeros((W, EP1, 128), BF16)           # host-expanded x4[src]
    ct1 = np.zeros((W, 4, EP1), np.float32)
    dlv1 = np.full((W, EP1), -10000.0, np.float32)

    ckey = o_core * NW + o_w
    gs = np.flatnonzero(np.r_[True, ckey[1:] != ckey[:-1]])
    ge = np.r_[gs[1:], len(ckey)]
    for a, b in zip(gs, ge):
        c = int(o_core[a]); ww = int(o_w[a])
        o = int(coff1[ww]); n = b - a
        g1[c, o:o + n] = x4[o_tr[a:b]]
        ct1[c, 0, o:o + n] = 1.0
        ct1[c, 1:4, o:o + n] = o_ea[a:b].T
        dlv1[c, o:o + n] = o_dl[a:b].astype(np.float32)

    g1 = np.ascontiguousarray(g1.transpose(0, 2, 1))     # [W, 128, EP1]
    # expand c rows 32x into the 128-wide kron layout (pure replication)
    ct1 = np.ascontiguousarray(
        np.repeat(ct1, 32, axis=1)).astype(BF16)         # [W, 128, EP1]

    dsf1 = np.zeros((W, 128, max(nb1, 1)), np.float32)
    for k in range(CH1):
        for (bcol, ws, items) in cb1[k]:
            dsf1[:, :, bcol] = dlv1[:, k * 128:(k + 1) * 128] - 128.0 * ws

    # ===== layer 2 layout: cells = (src quarter, dst window) ==============
    key2 = (core * 4 + q) * NW + w
    cnt2 = np.bincount(key2, minlength=W * 4 * NW).reshape(W, 4 * NW)
    # pad each quarter's edge total to a 512 multiple (extend last cell)
    R2 = cnt2.max(axis=0).astype(np.int64).reshape(4, NW)
    for qq in range(4):
        tot = int(R2[qq].sum())
        R2[qq, NW - 1] += _ceil(tot, 512) - tot
    qsz = R2.sum(axis=1)
    qoff = np.concatenate([[0], np.cumsum(qsz)])
    EP2 = int(qsz.sum())
    CH2 = EP2 // 128
    coff2 = np.zeros((4, NW), np.int64)
    run = 0
    cell_list2 = []
    for qq in range(4):
        for ww in range(NW):
            coff2[qq, ww] = run
            cell_list2.append((ww, run, int(R2[qq, ww])))
            run += int(R2[qq, ww])
    assert run == EP2
    nb2, cb2, lp2, wp2 = _pairs(cell_list2, CH2, NW)

    calls = []                                   # (q, edge_off, size)
    for qq in range(4):
        o = 0
        while o < qsz[qq]:
            s = min(2048, int(qsz[qq]) - o)
            calls.append((qq, int(qoff[qq]) + o, s))
            o += s

    order2 = np.lexsort((dl, w, q, core))
    s_core = core[order2]
    s_q = q[order2]
    s_w = w[order2]
    s_tr = tr[order2]
    s_dl = dl[order2]
    s_ea = ea[order2]

    gidx = np.zeros((W, EP2), np.int64)
    ct2 = np.zeros((W, 4, EP2), np.float32)
    dlv2 = np.full((W, EP2), -10000.0, np.float32)

    ckey2 = (s_core * 4 + s_q) * NW + s_w
    gs = np.flatnonzero(np.r_[True, ckey2[1:] != ckey2[:-1]])
    ge = np.r_[gs[1:], len(ckey2)]
    for a, b in zip(gs, ge):
        c = int(s_core[a]); qq = int(s_q[a]); ww = int(s_w[a])
        o = int(coff2[qq, ww]); n = b - a
        gidx[c, o:o + n] = s_tr[a:b] - qq * QS
        ct2[c, 0, o:o + n] = 1.0
        ct2[c, 1:4, o:o + n] = s_ea[a:b].T
        dlv2[c, o:o + n] = s_dl[a:b].astype(np.float32)

    gidx16 = np.stack([wrap_idx16(gidx[c]) for c in range(W)])
    ct2 = np.ascontiguousarray(
        np.repeat(ct2, 32, axis=1)).astype(BF16)         # [W, 128, EP2]

    dsf2 = np.zeros((W, 128, max(nb2, 1)), np.float32)
    for k in range(CH2):
        for (bcol, ws, items) in cb2[k]:
            dsf2[:, :, bcol] = dlv2[:, k * 128:(k + 1) * 128] - 128.0 * ws

    # x_augT packed (per core): window w at [32*(w%3):+4, (w//3)*128:+128]
    XCOLS = _ceil(NW, 3) // 3 * 128
    xaug = np.zeros((W, 128, XCOLS), np.float32)
    for c in range(W):
        xa = np.zeros((4, NLOCP), np.float32)
        xa[:DIM, :NLOC] = x[c * NLOC:(c + 1) * NLOC].T
        xa[3, :NLOC] = 1.0
        for ww in range(NW):
            xaug[c, 32 * (ww % 3):32 * (ww % 3) + 4,
                 (ww // 3) * 128:(ww // 3) * 128 + 128] = \
                xa[:, ww * 128:(ww + 1) * 128]

    # weights
    def stack_w(Wn, bn, in_c):
        S = np.zeros((128, HID), np.float32)
        B = bn.reshape(in_c, HID)
        S[0:in_c] = B
        for d in range(3):
            S[32 * (d + 1):32 * (d + 1) + in_c] = Wn[d].reshape(in_c, HID)
        return S.astype(BF16)

    w1stack = stack_w(np.asarray(params['Wn1'], np.float32),
                      np.asarray(params['bn1'], np.float32), DIM)
    w2stack = stack_w(np.asarray(params['Wn2'], np.float32),
                      np.asarray(params['bn2'], np.float32), HID)
    root1a = np.concatenate([np.asarray(params['root1'], np.float32),
                             np.asarray(params['b1'], np.float32)[None]], 0)
    root1a_p = np.zeros((68, HID), np.float32)
    for g in range(3):
        root1a_p[32 * g:32 * g + DIM] = root1a[:DIM]
        root1a_p[32 * g + 3] = root1a[DIM]
    root2a = np.concatenate([np.asarray(params['root2'], np.float32),
                             np.asarray(params['b2'], np.float32)[None]],
                            0).astype(BF16)
    wf1a = np.concatenate([np.asarray(params['Wf1'], np.float32),
                           np.asarray(params['bf1'], np.float32)[None]],
                          0).astype(BF16)
    wf2a = np.concatenate([np.asarray(params['Wf2'], np.float32),
                           np.asarray(params['bf2'], np.float32)[None]],
                          0).astype(BF16)
    iota = np.tile(np.arange(256, dtype=np.float32)[None, :],
                   (128, 1)).astype(BF16)
    ident = np.eye(128, dtype=np.float32)

    meta = dict(NLOC=NLOC, NLOCP=NLOCP, NP=NP, QS=QS, NW=NW,
                EP1=EP1, CH1=CH1, TB1=TB1, nb1=nb1, cb1=cb1, lp1=lp1, wp1=wp1,
                EP2=EP2, CH2=CH2, nb2=nb2, cb2=cb2, lp2=lp2, wp2=wp2,
                coff1=[int(v) for v in coff1], R1=[int(v) for v in R1],
                qoff2=[int(v) for v in qoff],
                XCOLS=XCOLS, calls=calls, W=W, HID=HID, DIM=DIM)

    shared = dict(w1stack=w1stack, w2stack=w2stack,
                  root1a=root1a_p, root2a=root2a, wf1a=wf1a, wf2a=wf2a,
                  iota=iota, ident=ident)
    in_maps = []
    for c in range(W):
        m = dict(shared)
        m['g1'] = g1[c]
        m['ct1'] = ct1[c]
        m['dsf1'] = dsf1[c]
        m['gidx'] = gidx16[c]
        m['ct2'] = ct2[c]
        m['dsf2'] = dsf2[c]
        m['xaug'] = xaug[c]
        in_maps.append(m)
    return in_maps, meta


def build_bass(meta):
    W, HID = meta['W'], meta['HID']
    NLOCP, NP, QS, NW = meta['NLOCP'], meta['NP'], meta['QS'], meta['NW']
    EP1, CH1, nb1 = meta['EP1'], meta['CH1'], meta['nb1']
    EP2, CH2, nb2 = meta['EP2'], meta['CH2'], meta['nb2']
    cb1, lp1, wp1 = meta['cb1'], meta['lp1'], meta['wp1']
    cb2, lp2, wp2 = meta['cb2'], meta['lp2'], meta['wp2']
    XCOLS, calls = meta['XCOLS'], meta['calls']
    coff1, R1, qoff2 = meta['coff1'], meta['R1'], meta['qoff2']
    RANKS_Q = QS // 128
    f32, bf16, i16 = mybir.dt.float32, mybir.dt.bfloat16, mybir.dt.int16

    nc = bacc.Bacc("TRN2", target_bir_lowering=False, debug=False,
                   num_devices=W, enable_asserts=False)

    # I/O ------------------------------------------------------------------
    g1_d = nc.dram_tensor("g1", [128, EP1], bf16, kind="ExternalInput")
    ct1_d = nc.dram_tensor("ct1", [128, EP1], bf16, kind="ExternalInput")
    dsf1_d = nc.dram_tensor("dsf1", [128, max(nb1, 1)], f32,
                            kind="ExternalInput")
    gidx_d = nc.dram_tensor("gidx", [128, EP2 // 16], i16,
                            kind="ExternalInput")
    ct2_d = nc.dram_tensor("ct2", [128, EP2], bf16, kind="ExternalInput")
    dsf2_d = nc.dram_tensor("dsf2", [128, max(nb2, 1)], f32,
                            kind="ExternalInput")
    xaug_d = nc.dram_tensor("xaug", [128, XCOLS], f32, kind="ExternalInput")
    w1_d = nc.dram_tensor("w1stack", [128, HID], bf16, kind="ExternalInput")
    w2_d = nc.dram_tensor("w2stack", [128, HID], bf16, kind="ExternalInput")
    r1_d = nc.dram_tensor("root1a", [68, HID], f32, kind="ExternalInput")
    r2_d = nc.dram_tensor("root2a", [33, HID], bf16, kind="ExternalInput")
    wf1_d = nc.dram_tensor("wf1a", [33, HID], bf16, kind="ExternalInput")
    wf2_d = nc.dram_tensor("wf2a", [33, 1], bf16, kind="ExternalInput")
    iota_d = nc.dram_tensor("iota", [128, 256], bf16, kind="ExternalInput")
    id_d = nc.dram_tensor("ident", [128, 128], f32, kind="ExternalInput")
    out_d = nc.dram_tensor("out", [1, NLOCP], f32, kind="ExternalOutput")

    cc_in = nc.dram_tensor("cc_in", [NLOCP, HID], bf16)
    cc_out = nc.dram_tensor("cc_out", [NP, HID], bf16, addr_space="Shared")

    ctx = ExitStack()
    with tile.TileContext(nc) as tc:
      with ctx:
        const = ctx.enter_context(tc.tile_pool(name="const", bufs=1))
        big = ctx.enter_context(tc.tile_pool(name="big", bufs=1))
        pipe = ctx.enter_context(tc.tile_pool(name="pipe", bufs=2))
        ohp = ctx.enter_context(tc.tile_pool(name="ohp", bufs=12))
        tabp = ctx.enter_context(tc.tile_pool(name="tabp", bufs=1))

        nc.gpsimd.load_library(library_config.mlp)

        # ---- constant loads ----
        def load(pool, dram, shape, dtype):
            t = pool.tile(shape, dtype, tag="c_" + dram.name)
            nc.sync.dma_start(out=t[:], in_=dram[:, :])
            return t

        dsf1_s = load(const, dsf1_d, [128, max(nb1, 1)], f32)
        gidx_s = load(const, gidx_d, [128, EP2 // 16], i16)
        dsf2_s = load(const, dsf2_d, [128, max(nb2, 1)], f32)
        xaug_s = load(const, xaug_d, [128, XCOLS], f32)
        w1_s = load(const, w1_d, [128, HID], bf16)
        w2_s = load(const, w2_d, [128, HID], bf16)
        r1_s = load(const, r1_d, [68, HID], f32)
        r2_s = load(const, r2_d, [33, HID], bf16)
        wf1_s = load(const, wf1_d, [33, HID], bf16)
        wf2_s = load(const, wf2_d, [33, 1], bf16)
        iota_s = load(const, iota_d, [128, 256], bf16)
        id_s = load(const, id_d, [128, 128], f32)

        CHMAX = max(CH1, CH2)
        msg_s = big.tile([128, CHMAX * 32], bf16)   # all msgs of one layer
        h1c_s = big.tile([128, NW * 32], bf16)      # compact local h1
        h1T_s = big.tile([33, NLOCP], bf16)         # h1^T augmented
        h2T_s = h1T_s                               # reused for h2^T (WAR-safe)
        nc.vector.memset(h1T_s[32:33, :], 1.0)

        def do_msg(ps1, t, Gt, Ct, b, wstack):
            """One 512-edge block of pass 1: messages into msg_s."""
            Ft = pipe.tile([128, 512], bf16, tag="F")
            nc.vector.tensor_tensor(Ft[:], Gt[:, 512 * b:512 * b + 512],
                                    Ct[:, 512 * b:512 * b + 512], ALU.mult)
            Mp = ps1.tile([128, 128], f32, tag="M")
            for j in range(4):
                nc.tensor.matmul(Mp[:, 32 * j:32 * j + 32],
                                 Ft[:, 128 * j:128 * j + 128],
                                 wstack[:], start=True, stop=True)
            nc.scalar.activation(msg_s[:, t * 128:(t + 1) * 128],
                                 Mp[:], AF.Copy)

        def do_agg(aggp, t, cbuilds, lastp, dsf_s):
            """Inline aggregation of block t's four chunks."""
            for kk in range(4):
                k = 4 * t + kk
                for (bcol, ws, items) in cbuilds[k]:
                    wd = 128 * len(items)
                    oh = ohp.tile([128, 256], bf16, tag="oh")
                    nc.vector.tensor_scalar(oh[:, 0:wd], iota_s[:, 0:wd],
                                            dsf_s[:, bcol:bcol + 1], 0.0,
                                            ALU.is_equal, ALU.bypass)
                    for (ww, sl) in items:
                        a = aggp[:, 32 * ww:32 * ww + 32]
                        nc.tensor.matmul(
                            a, oh[:, 128 * sl:128 * sl + 128],
                            msg_s[:, 32 * k:32 * k + 32], start=False,
                            stop=(lastp[ww] == (k, bcol, sl)),
                            skip_group_check=True)

        def roots(layer, aggp, winp):
            for ww in range(NW):
                if layer == 1:
                    g = 32 * (ww % 3)
                    lhs = xaug_s[g:g + 4,
                                 (ww // 3) * 128:(ww // 3) * 128 + 128]
                    rr = r1_s[g:g + 4, :]
                else:
                    lhs = h1T_s[:, ww * 128:(ww + 1) * 128]
                    rr = r2_s[:]
                a = aggp[:, 32 * ww:32 * ww + 32]
                nc.tensor.matmul(a, lhs, rr, start=False,
                                 stop=(winp[ww] == 0), skip_group_check=True)


        def make_bywin(cbuilds, CH, klo=0, khi=None):
            khi = CH if khi is None else khi
            bywin = [[] for _ in range(NW)]
            for k in range(klo, khi):
                for (bcol, ws, items) in cbuilds[k]:
                    for (ww, sl) in items:
                        bywin[ww].append((k, bcol, sl, 128 * len(items)))
            return bywin

        def emit_root(layer, aggp, ww, stop):
            if layer == 1:
                g = 32 * (ww % 3)
                lhs = xaug_s[g:g + 4,
                             (ww // 3) * 128:(ww // 3) * 128 + 128]
                rr = r1_s[g:g + 4, :]
            else:
                lhs = h1T_s[:, ww * 128:(ww + 1) * 128]
                rr = r2_s[:]
            a = aggp[:, 32 * ww:32 * ww + 32]
            nc.tensor.matmul(a, lhs, rr, start=False, stop=stop,
                             skip_group_check=True)

        def emit_win(aggp, ww, plist, dsf_s, lastp):
            a = aggp[:, 32 * ww:32 * ww + 32]
            for (k, bcol, sl, wd) in plist:
                oh = ohp.tile([128, 256], bf16, tag="oh")
                nc.vector.tensor_scalar(oh[:, 0:wd], iota_s[:, 0:wd],
                                        dsf_s[:, bcol:bcol + 1], 0.0,
                                        ALU.is_equal, ALU.bypass)
                nc.tensor.matmul(a, oh[:, 128 * sl:128 * sl + 128],
                                 msg_s[:, 32 * k:32 * k + 32],
                                 start=False,
                                 stop=(lastp[ww] == (k, bcol, sl)),
                                 skip_group_check=True)

        def tail_a(layer, aggp, ps):
            """relu + transpose into hT (and h1c on layer 1)."""
            hT = h1T_s if layer == 1 else h2T_s
            for w0 in range(0, NW, 4):
                nsub = min(4, NW - w0)
                trp = ps.tile([32, 512], f32, tag="tr")
                for i in range(nsub):
                    ww = w0 + i
                    a = aggp[:, 32 * ww:32 * ww + 32]
                    hf = pipe.tile([128, 32], f32, tag="hf")
                    nc.scalar.activation(hf[:], a, AF.Relu)
                    nc.tensor.transpose(trp[:, 128 * i:128 * i + 128],
                                        hf[:], id_s[:])
                span = 128 * nsub
                nc.scalar.activation(hT[0:32, 128 * w0:128 * w0 + span],
                                     trp[:, 0:span], AF.Copy)

        def tail_fc():
            with tc.tile_pool(name="psf", bufs=2, space="PSUM") as psf:
                for w0 in range(0, NW, 4):
                    span = 128 * min(4, NW - w0)
                    f1 = psf.tile([32, 512], f32, tag="f1")
                    nc.tensor.matmul(f1[:, 0:span], wf1_s[:],
                                     h2T_s[:, 128 * w0:128 * w0 + span],
                                     start=True, stop=True)
                    h3t = pipe.tile([33, 512], bf16, tag="h3t")
                    nc.scalar.activation(h3t[0:32, 0:span],
                                         f1[:, 0:span], AF.Relu)
                    nc.vector.memset(h3t[32:33, 0:span], 1.0)
                    f2 = psf.tile([1, 512], f32, tag="f2")
                    nc.tensor.matmul(f2[:, 0:span], wf2_s[:],
                                     h3t[:, 0:span], start=True, stop=True)
                    ot = pipe.tile([1, 512], f32, tag="ot")
                    nc.scalar.activation(ot[:, 0:span], f2[:, 0:span],
                                         AF.Copy)
                    nc.sync.dma_start(
                        out=out_d[:, 128 * w0:128 * w0 + span],
                        in_=ot[:, 0:span])

        # ================= layer 1 =================
        with nc.named_scope("l1"), \
             tc.tile_pool(name="agg1", bufs=1, space="PSUM") as pa1:
            aggp = pa1.tile([128, NW * 32], f32, tag="agg")
            nc.vector.memset(aggp[:], 0.0)
            bywin1 = make_bywin(cb1, CH1)
            sched1 = {}
            for ww in range(NW):
                lb = (coff1[ww] + R1[ww] - 1) // 512
                sched1.setdefault(lb, []).append(ww)
            with tc.tile_pool(name="ps1", bufs=1, space="PSUM") as ps1:
                for eoff in range(0, EP1, 2048):
                    csz = min(2048, EP1 - eoff)
                    Gt = pipe.tile([128, 2048], bf16, tag="G")
                    nc.sync.dma_start(out=Gt[:, 0:csz],
                                      in_=g1_d[:, eoff:eoff + csz])
                    Ct = pipe.tile([128, 2048], bf16, tag="C4")
                    nc.sync.dma_start(out=Ct[:, 0:csz],
                                      in_=ct1_d[:, eoff:eoff + csz])
                    for b in range(csz // 512):
                        t = (eoff + b * 512) // 512
                        do_msg(ps1, t, Gt, Ct, b, w1_s)
                        for ww in sched1.get(t, []):
                            emit_root(1, aggp, ww, wp1[ww] == 0)
                            emit_win(aggp, ww, bywin1[ww], dsf1_s, lp1)
                            nc.scalar.activation(
                                h1c_s[:, 32 * ww:32 * ww + 32],
                                aggp[:, 32 * ww:32 * ww + 32], AF.Relu)
            # ship compact h1 (deps: inline h1c copies only)
            nc.sync.dma_start(
                out=cc_in.ap().rearrange("(w p) h -> p w h", p=128),
                in_=h1c_s[:].rearrange("p (w h) -> p w h", h=HID))
            with tc.tile_pool(name="pst1", bufs=1, space="PSUM") as pst:
                tail_a(1, aggp, pst)

        with nc.named_scope("allgather"):
            nc.gpsimd.collective_compute(
                "AllGather", ALU.bypass, replica_groups=[list(range(W))],
                ins=[cc_in.ap().opt()], outs=[cc_out.ap().opt()])

        # ================= layer 2 =================
        with nc.named_scope("l2"), \
             tc.tile_pool(name="agg2", bufs=1, space="PSUM") as pa2:
            aggp = pa2.tile([128, NW * 32], f32, tag="agg")
            nc.vector.memset(aggp[:], 0.0)
            bywin2q = [make_bywin(cb2, CH2, qoff2[q] // 128,
                                  qoff2[q + 1] // 128) for q in range(4)]
            with tc.tile_pool(name="ps1b", bufs=1, space="PSUM") as ps1b:
                for qq in range(4):
                    cw = tabp.tile([128, RANKS_Q * 32], bf16, tag="cw")
                    nc.sync.dma_start(
                        out=cw[:].rearrange("p (r h) -> p r h", h=HID),
                        in_=cc_out.ap().rearrange("(q r p) h -> q p r h",
                                                  q=4, p=128)[qq])
                    tq = tabp.tile([128, RANKS_Q * 128], bf16, tag="tq")
                    tq4 = tq[:].rearrange("p (r d h) -> p r d h", d=4, h=HID)
                    cw3 = cw[:].rearrange("p (r h) -> p r h", h=HID)
                    for d in range(4):
                        nc.vector.tensor_copy(tq4[:, :, d, :], cw3)
                    for (cq, eoff, csz) in calls:
                        if cq != qq:
                            continue
                        Gt = pipe.tile([128, 2048], bf16, tag="G")
                        g3 = Gt[:, 0:csz].rearrange("p (o n) -> p o n", o=1)
                        nc.gpsimd.dma_gather(
                            g3, tq[:],
                            gidx_s[:, eoff // 16:(eoff + csz) // 16],
                            csz, csz, 128, transpose=True,
                            single_packet=False,
                            sbuf_tokens_per_rank=128,
                            sbuf_free_dim_per_rank=256,
                            sbuf_free_dim_pad_per_rank=0, sbuf_byte_offset=0)
                        Ct = pipe.tile([128, 2048], bf16, tag="C4")
                        nc.sync.dma_start(out=Ct[:, 0:csz],
                                          in_=ct2_d[:, eoff:eoff + csz])
                        for b in range(csz // 512):
                            t = (eoff + b * 512) // 512
                            do_msg(ps1b, t, Gt, Ct, b, w2_s)
                    if qq < 3:
                        for ww in range(NW):
                            if qq == 0:
                                emit_root(2, aggp, ww, wp2[ww] == 0)
                            emit_win(aggp, ww, bywin2q[qq][ww], dsf2_s, lp2)
            # final quarter's aggregation fused with the relu/transpose tail
            with tc.tile_pool(name="pst2", bufs=1, space="PSUM") as pst:
                for w0 in range(0, NW, 4):
                    nsub = min(4, NW - w0)
                    for i in range(nsub):
                        emit_win(aggp, w0 + i, bywin2q[3][w0 + i],
                                 dsf2_s, lp2)
                    trp = pst.tile([32, 512], f32, tag="tr")
                    for i in range(nsub):
                        ww = w0 + i
                        a = aggp[:, 32 * ww:32 * ww + 32]
                        hf = pipe.tile([128, 32], f32, tag="hf")
                        nc.scalar.activation(hf[:], a, AF.Relu)
                        nc.tensor.transpose(trp[:, 128 * i:128 * i + 128],
                                            hf[:], id_s[:])
                    span = 128 * nsub
                    nc.scalar.activation(
                        h2T_s[0:32, 128 * w0:128 * w0 + span],
                        trp[:, 0:span], AF.Copy)
        tail_fc()
    return nc


def run_kernel(inputs, cfg=None, trace=False):
    cfg = cfg or FULL_CFG
    W = cfg['W']
    params = {k: inputs[k] for k in
              ('Wn1', 'bn1', 'root1', 'b1', 'Wn2', 'bn2', 'root2', 'b2',
               'Wf1', 'bf1', 'Wf2', 'bf2')}
    in_maps, meta = host_prep(inputs['x'], inputs['edge_index'],
                              inputs['edge_attr'], params, cfg)
    nc = build_bass(meta)
    nc.finalize()
    res = run_bass_kernel_spmd(nc, in_maps, core_ids=list(range(W)),
                               trace=trace)
    NLOC = meta['NLOC']
    out = np.zeros((cfg['N'], 1), np.float32)
    for c in range(W):
        out[c * NLOC:(c + 1) * NLOC, 0] = res.results[c]['out'][0, :NLOC]
    return out, res


def kernel(**inputs):
    out, _ = run_kernel(inputs)
    return out
